# revision 18
# baseline (speedup 1.0000x reference)
"""FLARetNet Trainium2 kernel: 8-core SPMD, batch x head-group sharding.

Each core handles one batch (B=2 -> 4 cores per batch) and 4 of 16 heads.
Per core: qkvg projections (fp16 matmuls), neox RoPE, RetNet chunked
retention scan (chunk=256), fused RMSNorm + swish gate, output projection
(partial sum over its heads). Host sums the 4 partials per batch.

Schedule/layout notes:
- Retention decay is folded into the RoPE tables: q tables carry
  gamma^(i+1), k tables carry gamma^(-j-1) (i,j = position in chunk).
  Off-diagonal A blocks then need no mask at all; only the two 128x128
  diagonal blocks get a 0/1 triangular mask, and the strictly-upper
  block is never computed. kdkv becomes a constant gamma^C column scale.
- RoPE rotate-half runs as a DVE stream_shuffle: the q/k feature rows
  are permuted host-side (within each head's 64 dims) so rotation
  partners sit in the same 32-partition quadrant.
- Matmuls run fp16 (full PE rate); PSUM accumulation is fp32.
- The per-chunk Wo projection is deferred by one chunk and split into
  per-og-pair passes so it never waits on the full norm/gate chain.
- The rsqrt activation table is pre-warmed with a dummy op after each
  projection tile's silus so the table load stays off the norm chain.
"""
import numpy as np
import ml_dtypes

import concourse.mybir as mybir
import concourse.tile as tile
import concourse.bacc as bacc
import concourse.bass_isa as bass_isa
from concourse.bass_utils import run_bass_kernel_spmd

F32 = mybir.dt.float32
BF16 = mybir.dt.float16
AF = mybir.ActivationFunctionType
BF = np.float16

B, T, D, H = 2, 4096, 1024, 16
DK, DV = 64, 128
C = 256            # attention chunk length (math-equivalent for any C)
PT = 512           # projection token-tile
NCH = T // C       # 16 chunks
HPC = 4            # heads per core
NCORES = 8

# rope-partner shuffle: within each 32-partition quadrant swap halves
SHUF = list(range(16, 32)) + list(range(16))
# row permutation within each 64-dim head block so partners share a quadrant
P64 = list(range(16)) + list(range(32, 48)) + list(range(16, 32)) + list(range(48, 64))

_cache = {}


def _build_program():
    nc = bacc.Bacc("TRN2", target_bir_lowering=False, debug=False)

    XT = nc.dram_tensor("XT", [D, T], BF16, kind="ExternalInput")
    WQ = nc.dram_tensor("WQ", [128, 8, 256], BF16, kind="ExternalInput")
    WK = nc.dram_tensor("WK", [128, 8, 256], BF16, kind="ExternalInput")
    WV = nc.dram_tensor("WV", [128, 8, 512], BF16, kind="ExternalInput")
    WG = nc.dram_tensor("WG", [128, 8, 512], BF16, kind="ExternalInput")
    WO = nc.dram_tensor("WO", [128, 4, 1024], BF16, kind="ExternalInput")
    CSQ = nc.dram_tensor("CSQ", [128, 4, T], BF16, kind="ExternalInput")
    CSK = nc.dram_tensor("CSK", [128, 4, T], BF16, kind="ExternalInput")
    JT = nc.dram_tensor("JT", [128, 2, 256], BF16, kind="ExternalInput")
    GKC = nc.dram_tensor("GKC", [128, 256], F32, kind="ExternalInput")
    GCV = nc.dram_tensor("GCV", [128, 2], F32, kind="ExternalInput")
    IDENT = nc.dram_tensor("IDENT", [128, 128], BF16, kind="ExternalInput")
    ONES = nc.dram_tensor("ONES", [128, 1], BF16, kind="ExternalInput")
    ZS = nc.dram_tensor("ZS", [128, 2, C], BF16, kind="ExternalInput")

    OUT = nc.dram_tensor("OUT", [T, D], BF16, kind="ExternalOutput")

    with tile.TileContext(nc) as tc:
        with tc.tile_pool(name="singles", bufs=1) as singles, \
             tc.tile_pool(name="xt", bufs=2) as xt_pool, \
             tc.tile_pool(name="tab", bufs=2) as tab_pool, \
             tc.tile_pool(name="rope", bufs=2) as rope_pool, \
             tc.tile_pool(name="qk", bufs=2) as qk_pool, \
             tc.tile_pool(name="vsb", bufs=2) as v_pool, \
             tc.tile_pool(name="asb", bufs=3) as a_pool, \
             tc.tile_pool(name="gat", bufs=2) as g_pool, \
             tc.tile_pool(name="nrm", bufs=6) as nrm_pool, \
             tc.tile_pool(name="og", bufs=6) as og_pool, \
             tc.tile_pool(name="osb", bufs=3) as out_pool, \
             tc.tile_pool(name="ps_proj", bufs=2, space="PSUM") as ps_proj, \
             tc.tile_pool(name="ps_small", bufs=3, space="PSUM") as ps_small, \
             tc.tile_pool(name="ps_o", bufs=3, space="PSUM") as ps_o:

            # ---- resident weights/tables (usage order; wo last) ----
            wq = singles.tile([128, 8, 256], BF16)
            wk = singles.tile([128, 8, 256], BF16)
            wv = singles.tile([128, 8, 512], BF16)
            wg = singles.tile([128, 8, 512], BF16)
            wo = singles.tile([128, 4, 1024], BF16)
            identmm = singles.tile([128, 128], BF16)
            jtm = singles.tile([128, 2, 256], BF16)
            gkc = singles.tile([128, 256], F32)
            gcv = singles.tile([128, 2], F32)
            ones = singles.tile([128, 1], BF16)
            s_sb = singles.tile([128, 2, C], BF16)

            nc.sync.dma_start(out=wq[:, :, 0:128], in_=WQ[:, :, 0:128])
            nc.sync.dma_start(out=wq[:, :, 128:256], in_=WQ[:, :, 128:256])
            nc.gpsimd.dma_start(out=wk, in_=WK[:, :, :])
            nc.gpsimd.dma_start(out=wv, in_=WV[:, :, :])
            nc.gpsimd.dma_start(out=wg, in_=WG[:, :, :])
            nc.gpsimd.dma_start(out=identmm, in_=IDENT[:, :])
            nc.gpsimd.dma_start(out=jtm, in_=JT[:, :, :])
            nc.gpsimd.dma_start(out=gkc, in_=GKC[:, :])
            nc.gpsimd.dma_start(out=gcv, in_=GCV[:, :])
            nc.gpsimd.dma_start(out=ones, in_=ONES[:, :])
            nc.gpsimd.dma_start(out=s_sb, in_=ZS[:, :, :])
            nc.gpsimd.dma_start(out=wo, in_=WO[:, :, :])

            epsb = singles.tile([1, 1], F32)
            nc.vector.memset(epsb, 1e-5)

            def emit_wo(og_pair_list, oc0, last=False):
                for tb in range(2):
                    out_ps = [ps_small.tile([128, 512], F32, tag="small",
                                            name=f"wo{oc0}_{tb}_{nn}")
                              for nn in range(2)]
                    # pair-0 heads first so this can start before pair-1's
                    # norm chain finishes
                    for hh in range(2):
                        for nn in range(2):
                            nc.tensor.matmul(
                                out_ps[nn],
                                lhsT=og_pair_list[0][
                                    :, hh, tb * 128:(tb + 1) * 128],
                                rhs=wo[:, hh, nn * 512:(nn + 1) * 512],
                                start=(hh == 0), stop=False)
                    for hh in range(2):
                        for nn in range(2):
                            nc.tensor.matmul(
                                out_ps[nn],
                                lhsT=og_pair_list[1][
                                    :, hh, tb * 128:(tb + 1) * 128],
                                rhs=wo[:, 2 + hh, nn * 512:(nn + 1) * 512],
                                start=False, stop=(hh == 1))
                    for nn in range(2):
                        out_sb = out_pool.tile([128, 512], BF16, tag="outsb",
                                               name=f"wos{oc0}_{tb}_{nn}")
                        if last and nn == 1:
                            nc.vector.tensor_copy(out=out_sb, in_=out_ps[nn])
                        else:
                            nc.scalar.copy(out_sb, out_ps[nn])
                        nc.sync.dma_start(
                            out=OUT[oc0 + tb * 128:oc0 + (tb + 1) * 128,
                                    nn * 512:(nn + 1) * 512],
                            in_=out_sb)

            pending_wo = []

            def norm_tail_p(p, osc, rsqf, gsil_t, cch, ccc):
                onrm = nrm_pool.tile([128, 512], F32, tag="onrm",
                                     name=f"onrm{cch}_{p}")
                nc.vector.tensor_mul(onrm, osc, rsqf)
                og = og_pool.tile([128, 2, C], BF16, tag="og",
                                  name=f"og{cch}_{p}")
                gs = gsil_t[:, p * 2:(p + 1) * 2, ccc * C:(ccc + 1) * C]
                nc.vector.tensor_mul(
                    og, onrm.rearrange('p (a b) -> p a b', a=2), gs)
                return og

            for pt in range(T // PT):
                p0 = pt * PT

                xt = xt_pool.tile([128, 8, PT], BF16, tag="xt")
                xt_src = XT.rearrange("(db p) t -> p db t", p=128)
                for xq in range(4):
                    eng = nc.sync if xq % 2 == 0 else nc.scalar
                    eng.dma_start(out=xt[:, 2 * xq:2 * xq + 2, :],
                                  in_=xt_src[:, 2 * xq:2 * xq + 2,
                                             p0:p0 + PT])

                csq = tab_pool.tile([128, 4, PT], BF16, tag="csq")
                nc.sync.dma_start(out=csq, in_=CSQ[:, :, p0:p0 + PT])
                csk = tab_pool.tile([128, 4, PT], BF16, tag="csk")
                nc.scalar.dma_start(out=csk, in_=CSK[:, :, p0:p0 + PT])

                # ---- projections over PT tokens ----
                # q, k feature-major [128(dim%128), blk, tok]; RoPE fused and
                # the retention decay folded into the cos/sin tables
                def proj_rope(w, cs, tag):
                    out = qk_pool.tile([128, 2, PT], BF16, tag=tag,
                                       name=f"{tag}{pt}")
                    for m in range(2):
                        pps = ps_proj.tile([128, PT], F32, tag="proj",
                                           name=f"{tag}ps{pt}_{m}")
                        for db in range(8):
                            nc.tensor.matmul(
                                pps, lhsT=w[:, db, m * 128:(m + 1) * 128],
                                rhs=xt[:, db, :],
                                start=(db == 0), stop=(db == 7))
                        tcos = rope_pool.tile([128, PT], BF16, tag="tcos")
                        tsin = rope_pool.tile([128, PT], BF16, tag="tsin")
                        rot = rope_pool.tile([128, PT], BF16, tag="rot")
                        nc.vector.tensor_mul(tcos, pps, cs[:, m, :])
                        nc.vector.tensor_mul(tsin, pps, cs[:, 2 + m, :])
                        nc.vector.stream_shuffle(rot, tsin, SHUF)
                        nc.vector.tensor_add(out[:, m, :], tcos, rot)
                    return out

                q_sb = proj_rope(wq, csq, "q")   # gamma^(i+1) q, fp16
                k_sb = proj_rope(wk, csk, "k")   # gamma^(-j-1) k, fp16

                # v token-major [128(tok%128), tb, dim]
                v_sb = v_pool.tile([128, 4, 512], BF16, tag="v")
                for tb in range(4):
                    v_ps = ps_proj.tile([128, 512], F32, tag="proj",
                                        name=f"vps{pt}_{tb}")
                    for db in range(8):
                        nc.tensor.matmul(
                            v_ps,
                            lhsT=xt[:, db, tb * 128:(tb + 1) * 128],
                            rhs=wv[:, db, :],
                            start=(db == 0), stop=(db == 7))
                    nc.scalar.copy(v_sb[:, tb, :], v_ps)

                # g feature-major per head-block -> silu
                gsil = g_pool.tile([128, 4, PT], BF16, tag="gsil")
                for m in range(4):
                    g_ps = ps_proj.tile([128, PT], F32, tag="proj",
                                        name=f"gps{pt}_{m}")
                    for db in range(8):
                        nc.tensor.matmul(
                            g_ps, lhsT=wg[:, db, m * 128:(m + 1) * 128],
                            rhs=xt[:, db, :],
                            start=(db == 0), stop=(db == 7))
                    nc.scalar.activation(gsil[:, m, :], g_ps, AF.Silu)

                # ---- per 256-chunk attention ----
                for cc in range(PT // C):
                    ch = pt * (PT // C) + cc
                    c0 = ch * C
                    qs = q_sb[:, :, cc * C:(cc + 1) * C]
                    ks = k_sb[:, :, cc * C:(cc + 1) * C]
                    vtb0 = cc * 2

                    # k token-major + gamma^C scaling (for the state update)
                    ktm_ps = ps_small.tile([128, 2, C], BF16, tag="small",
                                           name=f"ktm{ch}")
                    for tb in range(2):
                        for b in range(2):
                            nc.tensor.transpose(
                                ktm_ps[:, tb, b * 128:(b + 1) * 128],
                                ks[:, b, tb * 128:(tb + 1) * 128],
                                identmm)
                    kdkv = qk_pool.tile([128, 2, C], BF16, tag="kdkv")
                    nc.vector.tensor_mul(
                        kdkv, ktm_ps,
                        gkc[:, None, :].broadcast_to([128, 2, C]))

                    o_ps_pairs = [ps_o.tile([128, 2, C], F32, tag="o",
                                            name=f"o_ps{ch}_{i}")
                                  for i in range(2)]
                    a_sbs = [None] * HPC

                    def emit_at(h):
                        blk, pb = h // 2, (h % 2) * 64
                        at_ps = ps_small.tile([128, 2, C], F32, tag="small",
                                              name=f"at{ch}_{h}")
                        nc.tensor.matmul(at_ps[:, 0, :],
                                         lhsT=ks[pb:pb + 64, blk, 0:128],
                                         rhs=qs[pb:pb + 64, blk, :],
                                         start=True, stop=True)
                        nc.tensor.matmul(at_ps[:, 1, 128:256],
                                         lhsT=ks[pb:pb + 64, blk, 128:256],
                                         rhs=qs[pb:pb + 64, blk, 128:256],
                                         start=True, stop=True)
                        a_sb = a_pool.tile([128, 2, C], BF16, tag="a",
                                           name=f"a{ch}_{h}")
                        nc.vector.tensor_mul(a_sb, at_ps, jtm)
                        a_sbs[h] = a_sb

                    def emit_o(h):
                        # intra-chunk matmuls first; the o_inter matmul last
                        # so the cross-chunk state dependency lands as late
                        # as possible in the PE queue
                        p, hh = h // 2, h % 2
                        blk, pb = h // 2, (h % 2) * 64
                        o_slice = o_ps_pairs[p][:, hh, :]
                        vl = v_sb[:, vtb0, h * 128:(h + 1) * 128]
                        a_sb = a_sbs[h]
                        if ch > 0:
                            # inter first, jb1, then one merged full-width
                            # jb0 matmul carrying the single stop (last
                            # writer of both psum halves)
                            nc.tensor.matmul(
                                o_slice,
                                lhsT=s_sb[hh * 64:hh * 64 + 64, p,
                                          hh * 128:(hh + 1) * 128],
                                rhs=qs[pb:pb + 64, blk, :],
                                start=True, stop=False)
                            nc.tensor.matmul(o_slice[:, 128:256],
                                             lhsT=v_sb[:, vtb0 + 1,
                                                       h * 128:(h + 1) * 128],
                                             rhs=a_sb[:, 1, 128:256],
                                             start=False, stop=False)
                            nc.tensor.matmul(o_slice, lhsT=vl,
                                             rhs=a_sb[:, 0, :],
                                             start=False, stop=True)
                        else:
                            nc.tensor.matmul(o_slice[:, 0:128], lhsT=vl,
                                             rhs=a_sb[:, 0, 0:128],
                                             start=True, stop=True)
                            nc.tensor.matmul(o_slice[:, 128:256], lhsT=vl,
                                             rhs=a_sb[:, 0, 128:256],
                                             start=True, stop=False)
                            nc.tensor.matmul(o_slice[:, 128:256],
                                             lhsT=v_sb[:, vtb0 + 1,
                                                       h * 128:(h + 1) * 128],
                                             rhs=a_sb[:, 1, 128:256],
                                             start=False, stop=True)

                    def norm_front(p):
                        """scalar/vector front half of the norm chain."""
                        o_ps = o_ps_pairs[p]
                        o_flat = o_ps.rearrange('p a b -> p (a b)')
                        osc = nrm_pool.tile([128, 512], F32, tag="osc",
                                            name=f"osc{ch}_{p}")
                        nc.scalar.copy(osc, o_flat)
                        o2 = nrm_pool.tile([128, 512], BF16, tag="o2",
                                           name=f"o2{ch}_{p}")
                        nc.vector.tensor_mul(o2, osc, o_flat)
                        return osc, o2

                    def norm_mid(p, osc, o2):
                        """mean matmul + rsqrt + partition broadcast."""
                        mean_ps = ps_small.tile([1, 512], F32, tag="small",
                                                name=f"mean{ch}_{p}")
                        nc.tensor.matmul(mean_ps, lhsT=ones, rhs=o2,
                                         start=True, stop=True)
                        rsq1 = nrm_pool.tile([1, 512], F32, tag="rsq1",
                                             name=f"rsq{ch}_{p}")
                        nc.scalar.activation(rsq1, mean_ps,
                                             AF.Abs_reciprocal_sqrt,
                                             bias=epsb, scale=1.0 / DV)
                        rsqf = nrm_pool.tile([128, 512], F32, tag="rsqf",
                                             name=f"rsqf{ch}_{p}")
                        nc.gpsimd.partition_broadcast(rsqf, rsq1)
                        return rsqf

                    emit_at(0)
                    emit_at(1)

                    # state-update matmuls early: inputs (kdkv, v) are ready
                    # at chunk start, and the s_sb write then lands well
                    # before the next chunk's o_inter reads it
                    ds_ps = ps_small.tile([128, 2, C], F32, tag="small",
                                          name=f"ds{ch}")
                    for p in range(2):
                        for jb in range(2):
                            nc.tensor.matmul(
                                ds_ps[:, p, :],
                                lhsT=kdkv[:, jb, p * 128:(p + 1) * 128],
                                rhs=v_sb[:, vtb0 + jb, p * 256:(p + 1) * 256],
                                start=(jb == 0), stop=(jb == 1))

                    emit_o(0)
                    emit_at(2)
                    emit_o(1)            # pair 0 o_ps complete
                    nf0 = norm_front(0)
                    emit_at(3)
                    emit_o(2)
                    emit_o(HPC - 1)      # pair 1 o_ps complete

                    for p in range(2):
                        nc.vector.scalar_tensor_tensor(
                            out=s_sb[:, p, :],
                            in0=s_sb[:, p, :],
                            scalar=gcv[:, p:p + 1],
                            in1=ds_ps[:, p, :],
                            op0=mybir.AluOpType.mult,
                            op1=mybir.AluOpType.add)
                    nf1 = norm_front(1)
                    if ch == NCH - 1:
                        rsqf1 = norm_mid(1, *nf1)
                    rsqf0 = norm_mid(0, *nf0)

                    # deferred norm tail + output projection of older
                    # chunks (two-chunk deferral: og is never the limiter)
                    while (len(pending_wo) >= 2
                           or (pending_wo and ch == NCH - 1)):
                        pnf0, pnf1, prs0, prs1, pgsil, pch, pcc, pc0 = \
                            pending_wo.pop(0)
                        og_prev = [norm_tail_p(0, pnf0[0], prs0, pgsil,
                                               pch, pcc),
                                   norm_tail_p(1, pnf1[0], prs1, pgsil,
                                               pch, pcc)]
                        emit_wo(og_prev, pc0)
                    if ch != NCH - 1:
                        rsqf1 = norm_mid(1, *nf1)

                    if ch == NCH - 1:
                        og_pairs = [norm_tail_p(0, nf0[0], rsqf0, gsil,
                                                ch, cc),
                                    norm_tail_p(1, nf1[0], rsqf1, gsil,
                                                ch, cc)]
                        emit_wo(og_pairs, c0, last=True)
                    else:
                        pending_wo.append((nf0, nf1, rsqf0, rsqf1, gsil,
                                           ch, cc, c0))

    nc.finalize()
    return nc


def _host_tables(heads):
    """Per-core constant tables for a 4-head slice."""
    gam = (1.0 - 2.0 ** (-5.0 - np.arange(H, dtype=np.float64)))[heads]  # [4]

    # rope tables with retention decay folded in, feature-major [128, 4, T]
    # (cos m0, cos m1, sin m0, sin m1); rows permuted by P64 within each
    # 64-dim head block so rotate partners share a 32-partition quadrant
    inv = 10000.0 ** (-np.arange(0, DK, 2, dtype=np.float64) / DK)  # [32]
    t_idx = np.arange(T, dtype=np.float64)
    ang = np.outer(t_idx, inv)                      # [T, 32]
    cos_t, sin_t = np.cos(ang), np.sin(ang)         # [T, 32]
    i_in_chunk = (np.arange(T) % C).astype(np.float64)

    CSQ = np.empty((128, 4, T), np.float64)
    CSK = np.empty((128, 4, T), np.float64)
    for m in range(2):
        for p in range(128):
            f = m * 128 + p            # feature index within the 4-head slice
            hc = f // 64               # head-local index 0..3
            dd = P64[f % 64]           # original dim within the head
            idx = dd % 32
            sign = 1.0 if dd < 32 else -1.0
            dq = gam[hc] ** (i_in_chunk + 1.0)
            dk = gam[hc] ** (-i_in_chunk - 1.0)
            CSQ[p, m, :] = cos_t[:, idx] * dq
            CSQ[p, 2 + m, :] = sign * sin_t[:, idx] * dq
            CSK[p, m, :] = cos_t[:, idx] * dk
            CSK[p, 2 + m, :] = sign * sin_t[:, idx] * dk

    # full-tile A mask [128(j), 2(jb), 256(i)]: jb0 = [tri | ones]
    # (diagonal block then unmasked off-diagonal), jb1 = [zero | tri]
    # (the never-read strictly-upper quarter stays zeroed)
    j_idx = np.arange(128)
    tri = (j_idx[:, None] <= j_idx[None, :]).astype(np.float32)
    on = np.ones((128, 128), np.float32)
    JTt = np.stack([np.concatenate([tri, on], 1),
                    np.concatenate([0 * on, tri], 1)], axis=1)

    # gamma^C per k-token-major column (column c -> head c//64)
    GKCt = np.broadcast_to(
        np.repeat(gam ** C, 64)[None, :], (128, 256)).astype(np.float32)

    # gamma^C per state-pair row
    GCVt = np.empty((128, 2), np.float32)
    for p in range(2):
        GCVt[0:64, p] = gam[2 * p] ** C
        GCVt[64:128, p] = gam[2 * p + 1] ** C
    return CSQ, CSK, JTt, GKCt, GCVt


def _prepare_inputs(x, Wq, Wk, Wv, Wg, Wo, g_norm_w):
    x = np.asarray(x, np.float32)
    Wq = np.asarray(Wq, np.float32) * (DK ** -0.5)
    Wk = np.asarray(Wk, np.float32)
    Wv = np.asarray(Wv, np.float32)
    Wg = np.asarray(Wg, np.float32)
    Wo = np.asarray(Wo, np.float32)
    gw = np.asarray(g_norm_w, np.float32)

    in_maps = []
    for core in range(NCORES):
        b = core // 4
        hg = core % 4
        heads = np.arange(4 * hg, 4 * hg + 4)
        # q/k columns with the P64 row permutation applied per head
        qk_cols = np.concatenate(
            [h * DK + np.asarray(P64) for h in heads])
        vg_cols = np.concatenate([np.arange(h * DV, (h + 1) * DV) for h in heads])

        XTc = np.ascontiguousarray(x[b].T).astype(BF)
        WQc = np.ascontiguousarray(
            Wq[:, qk_cols].reshape(8, 128, 256).transpose(1, 0, 2)).astype(BF)
        WKc = np.ascontiguousarray(
            Wk[:, qk_cols].reshape(8, 128, 256).transpose(1, 0, 2)).astype(BF)
        WVc = np.ascontiguousarray(
            Wv[:, vg_cols].reshape(8, 128, 512).transpose(1, 0, 2)).astype(BF)
        WGc = np.ascontiguousarray(
            Wg[:, vg_cols].reshape(8, 128, 512).transpose(1, 0, 2)).astype(BF)
        WOc = np.ascontiguousarray(
            (Wo[vg_cols, :] * np.tile(gw, 4)[:, None])
            .reshape(4, 128, 1024).transpose(1, 0, 2)).astype(BF)

        CSQt, CSKt, JTt, GKCt, GCVt = _host_tables(heads)

        in_maps.append({
            "XT": XTc, "WQ": WQc, "WK": WKc, "WV": WVc, "WG": WGc, "WO": WOc,
            "CSQ": CSQt.astype(BF), "CSK": CSKt.astype(BF),
            "JT": JTt.astype(BF), "GKC": GKCt,
            "GCV": GCVt,
            "ONES": np.ones((128, 1), BF),
            "IDENT": np.eye(128, dtype=BF),
            "ZS": np.zeros((128, 2, C), BF),
        })
    return in_maps


def _run(in_maps, **kw):
    if "nc" not in _cache:
        _cache["nc"] = _build_program()
    return run_bass_kernel_spmd(_cache["nc"], in_maps,
                                core_ids=list(range(NCORES)), **kw)


def kernel(x, Wq, Wk, Wv, Wg, Wo, g_norm_w):
    in_maps = _prepare_inputs(x, Wq, Wk, Wv, Wg, Wo, g_norm_w)
    res = _run(in_maps)
    out = np.zeros((B, T, D), np.float32)
    for core in range(NCORES):
        out[core // 4] += res.results[core]["OUT"].astype(np.float32)
    return out


# revision 19
# speedup vs baseline: 1.1764x; 1.1764x over previous
"""FLARetNet Trainium2 kernel: 8-core SPMD, batch x head-group sharding.

Each core handles one batch (B=2 -> 4 cores per batch) and 4 of 16 heads.
Per core: qkvg projections (fp16 matmuls), neox RoPE, RetNet chunked
retention scan (chunk=256), fused RMSNorm + swish gate, output projection
(partial sum over its heads). Host sums the 4 partials per batch.

Schedule/layout notes:
- Retention decay is folded into the RoPE tables: q tables carry
  gamma^(i+1), k tables carry gamma^(-j-1) (i,j = position in chunk).
  Off-diagonal A blocks then need no mask at all; only the two 128x128
  diagonal blocks get a 0/1 triangular mask, and the strictly-upper
  block is never computed. kdkv becomes a constant gamma^C column scale.
- RoPE rotate-half runs as a DVE stream_shuffle: the q/k feature rows
  are permuted host-side (within each head's 64 dims) so rotation
  partners sit in the same 32-partition quadrant.
- Matmuls run fp16 (full PE rate); PSUM accumulation is fp32.
- The per-chunk Wo projection is deferred by one chunk and split into
  per-og-pair passes so it never waits on the full norm/gate chain.
- The rsqrt activation table is pre-warmed with a dummy op after each
  projection tile's silus so the table load stays off the norm chain.
"""
import numpy as np
import ml_dtypes

import concourse.mybir as mybir
import concourse.tile as tile
import concourse.bacc as bacc
import concourse.bass_isa as bass_isa
from concourse.bass_utils import run_bass_kernel_spmd

F32 = mybir.dt.float32
BF16 = mybir.dt.float16
AF = mybir.ActivationFunctionType
BF = np.float16

B, T, D, H = 2, 4096, 1024, 16
DK, DV = 64, 128
C = 256            # attention chunk length (math-equivalent for any C)
PT = 512           # projection token-tile
NCH = T // C       # 16 chunks
HPC = 4            # heads per core
NCORES = 8

# rope-partner shuffle: within each 32-partition quadrant swap halves
SHUF = list(range(16, 32)) + list(range(16))
# row permutation within each 64-dim head block so partners share a quadrant
P64 = list(range(16)) + list(range(32, 48)) + list(range(16, 32)) + list(range(48, 64))

_cache = {}


def _build_program():
    nc = bacc.Bacc("TRN2", target_bir_lowering=False, debug=False)

    XT = nc.dram_tensor("XT", [D, T], BF16, kind="ExternalInput")
    WQ = nc.dram_tensor("WQ", [128, 8, 256], BF16, kind="ExternalInput")
    WK = nc.dram_tensor("WK", [128, 8, 256], BF16, kind="ExternalInput")
    WV = nc.dram_tensor("WV", [128, 8, 512], BF16, kind="ExternalInput")
    WG = nc.dram_tensor("WG", [128, 8, 512], BF16, kind="ExternalInput")
    WO = nc.dram_tensor("WO", [128, 4, 1024], BF16, kind="ExternalInput")
    CSQ = nc.dram_tensor("CSQ", [128, 4, T], BF16, kind="ExternalInput")
    CSK = nc.dram_tensor("CSK", [128, 4, T], BF16, kind="ExternalInput")
    JT = nc.dram_tensor("JT", [128, 2, 256], BF16, kind="ExternalInput")
    GKC = nc.dram_tensor("GKC", [128, 256], F32, kind="ExternalInput")
    GCV = nc.dram_tensor("GCV", [128, 2], F32, kind="ExternalInput")
    IDENT = nc.dram_tensor("IDENT", [128, 128], BF16, kind="ExternalInput")
    ONES = nc.dram_tensor("ONES", [128, 1], BF16, kind="ExternalInput")
    ZS = nc.dram_tensor("ZS", [128, 2, C], BF16, kind="ExternalInput")

    OUT = nc.dram_tensor("OUT", [T, D], BF16, kind="ExternalOutput")

    with tile.TileContext(nc) as tc:
        with tc.tile_pool(name="singles", bufs=1) as singles, \
             tc.tile_pool(name="xt", bufs=2) as xt_pool, \
             tc.tile_pool(name="tab", bufs=2) as tab_pool, \
             tc.tile_pool(name="rope", bufs=2) as rope_pool, \
             tc.tile_pool(name="qk", bufs=2) as qk_pool, \
             tc.tile_pool(name="vsb", bufs=2) as v_pool, \
             tc.tile_pool(name="asb", bufs=3) as a_pool, \
             tc.tile_pool(name="gat", bufs=2) as g_pool, \
             tc.tile_pool(name="nrm", bufs=6) as nrm_pool, \
             tc.tile_pool(name="og", bufs=6) as og_pool, \
             tc.tile_pool(name="osb", bufs=3) as out_pool, \
             tc.tile_pool(name="ps_proj", bufs=2, space="PSUM") as ps_proj, \
             tc.tile_pool(name="ps_small", bufs=3, space="PSUM") as ps_small, \
             tc.tile_pool(name="ps_o", bufs=3, space="PSUM") as ps_o:

            # ---- resident weights/tables (usage order; wo last) ----
            wq = singles.tile([128, 8, 256], BF16)
            wk = singles.tile([128, 8, 256], BF16)
            wv = singles.tile([128, 8, 512], BF16)
            wg = singles.tile([128, 8, 512], BF16)
            wo = singles.tile([128, 4, 1024], BF16)
            identmm = singles.tile([128, 128], BF16)
            jtm = singles.tile([128, 2, 256], BF16)
            gkc = singles.tile([128, 256], F32)
            gcv = singles.tile([128, 2], F32)
            ones = singles.tile([128, 1], BF16)
            s_sb = singles.tile([128, 2, C], BF16)

            nc.sync.dma_start(out=wq[:, :, 0:128], in_=WQ[:, :, 0:128])
            nc.sync.dma_start(out=wq[:, :, 128:256], in_=WQ[:, :, 128:256])
            nc.gpsimd.dma_start(out=wk, in_=WK[:, :, :])
            nc.gpsimd.dma_start(out=wv, in_=WV[:, :, :])
            nc.gpsimd.dma_start(out=wg, in_=WG[:, :, :])
            nc.gpsimd.dma_start(out=identmm, in_=IDENT[:, :])
            nc.gpsimd.dma_start(out=jtm, in_=JT[:, :, :])
            nc.gpsimd.dma_start(out=gkc, in_=GKC[:, :])
            nc.gpsimd.dma_start(out=gcv, in_=GCV[:, :])
            nc.gpsimd.dma_start(out=ones, in_=ONES[:, :])
            nc.gpsimd.dma_start(out=s_sb, in_=ZS[:, :, :])
            nc.gpsimd.dma_start(out=wo, in_=WO[:, :, :])

            epsb = singles.tile([1, 1], F32)
            nc.vector.memset(epsb, 1e-5)

            def emit_wo(og_pair_list, oc0, last=False):
                for tb in range(2):
                    out_ps = [ps_small.tile([128, 512], F32, tag="small",
                                            name=f"wo{oc0}_{tb}_{nn}")
                              for nn in range(2)]
                    # pair-0 heads first so this can start before pair-1's
                    # norm chain finishes
                    for hh in range(2):
                        for nn in range(2):
                            nc.tensor.matmul(
                                out_ps[nn],
                                lhsT=og_pair_list[0][
                                    :, hh, tb * 128:(tb + 1) * 128],
                                rhs=wo[:, hh, nn * 512:(nn + 1) * 512],
                                start=(hh == 0), stop=False)
                    for hh in range(2):
                        for nn in range(2):
                            nc.tensor.matmul(
                                out_ps[nn],
                                lhsT=og_pair_list[1][
                                    :, hh, tb * 128:(tb + 1) * 128],
                                rhs=wo[:, 2 + hh, nn * 512:(nn + 1) * 512],
                                start=False, stop=(hh == 1))
                    for nn in range(2):
                        out_sb = out_pool.tile([128, 512], BF16, tag="outsb",
                                               name=f"wos{oc0}_{tb}_{nn}")
                        if last and nn == 1:
                            nc.vector.tensor_copy(out=out_sb, in_=out_ps[nn])
                        else:
                            nc.scalar.copy(out_sb, out_ps[nn])
                        nc.sync.dma_start(
                            out=OUT[oc0 + tb * 128:oc0 + (tb + 1) * 128,
                                    nn * 512:(nn + 1) * 512],
                            in_=out_sb)

            pending_wo = []

            def norm_tail_p(p, osc, rsqf, gsil_t, cch, ccc):
                onrm = nrm_pool.tile([128, 512], F32, tag="onrm",
                                     name=f"onrm{cch}_{p}")
                nc.vector.tensor_mul(onrm, osc, rsqf)
                og = og_pool.tile([128, 2, C], BF16, tag="og",
                                  name=f"og{cch}_{p}")
                gs = gsil_t[:, p * 2:(p + 1) * 2, ccc * C:(ccc + 1) * C]
                nc.vector.tensor_mul(
                    og, onrm.rearrange('p (a b) -> p a b', a=2), gs)
                return og

            for pt in range(T // PT):
                p0 = pt * PT

                xt = xt_pool.tile([128, 8, PT], BF16, tag="xt")
                xt_src = XT.rearrange("(db p) t -> p db t", p=128)
                for xq in range(4):
                    eng = nc.sync if xq % 2 == 0 else nc.scalar
                    eng.dma_start(out=xt[:, 2 * xq:2 * xq + 2, :],
                                  in_=xt_src[:, 2 * xq:2 * xq + 2,
                                             p0:p0 + PT])

                csq = tab_pool.tile([128, 4, PT], BF16, tag="csq")
                nc.sync.dma_start(out=csq, in_=CSQ[:, :, p0:p0 + PT])
                csk = tab_pool.tile([128, 4, PT], BF16, tag="csk")
                nc.scalar.dma_start(out=csk, in_=CSK[:, :, p0:p0 + PT])

                # ---- projections over PT tokens ----
                # q, k feature-major [128(dim%128), blk, tok]; RoPE fused and
                # the retention decay folded into the cos/sin tables
                def proj_rope(w, cs, tag):
                    out = qk_pool.tile([128, 2, PT], BF16, tag=tag,
                                       name=f"{tag}{pt}")
                    for m in range(2):
                        pps = ps_proj.tile([128, PT], F32, tag="proj",
                                           name=f"{tag}ps{pt}_{m}")
                        for db in range(8):
                            nc.tensor.matmul(
                                pps, lhsT=w[:, db, m * 128:(m + 1) * 128],
                                rhs=xt[:, db, :],
                                start=(db == 0), stop=(db == 7))
                        tcos = rope_pool.tile([128, PT], BF16, tag="tcos")
                        tsin = rope_pool.tile([128, PT], BF16, tag="tsin")
                        rot = rope_pool.tile([128, PT], BF16, tag="rot")
                        nc.vector.tensor_mul(tcos, pps, cs[:, m, :])
                        nc.vector.tensor_mul(tsin, pps, cs[:, 2 + m, :])
                        nc.vector.stream_shuffle(rot, tsin, SHUF)
                        nc.vector.tensor_add(out[:, m, :], tcos, rot)
                    return out

                q_sb = proj_rope(wq, csq, "q")   # gamma^(i+1) q, fp16
                k_sb = proj_rope(wk, csk, "k")   # gamma^(-j-1) k, fp16

                # v token-major [128(tok%128), tb, dim]
                v_sb = v_pool.tile([128, 4, 512], BF16, tag="v")
                for tb in range(4):
                    v_ps = ps_proj.tile([128, 512], F32, tag="proj",
                                        name=f"vps{pt}_{tb}")
                    for db in range(8):
                        nc.tensor.matmul(
                            v_ps,
                            lhsT=xt[:, db, tb * 128:(tb + 1) * 128],
                            rhs=wv[:, db, :],
                            start=(db == 0), stop=(db == 7))
                    nc.scalar.copy(v_sb[:, tb, :], v_ps)

                # g feature-major per head-block -> silu
                gsil = g_pool.tile([128, 4, PT], BF16, tag="gsil")
                for m in range(4):
                    g_ps = ps_proj.tile([128, PT], F32, tag="proj",
                                        name=f"gps{pt}_{m}")
                    for db in range(8):
                        nc.tensor.matmul(
                            g_ps, lhsT=wg[:, db, m * 128:(m + 1) * 128],
                            rhs=xt[:, db, :],
                            start=(db == 0), stop=(db == 7))
                    nc.scalar.activation(gsil[:, m, :], g_ps, AF.Silu)

                # ---- per 256-chunk attention ----
                for cc in range(PT // C):
                    ch = pt * (PT // C) + cc
                    c0 = ch * C
                    qs = q_sb[:, :, cc * C:(cc + 1) * C]
                    ks = k_sb[:, :, cc * C:(cc + 1) * C]
                    vtb0 = cc * 2

                    # k token-major + gamma^C scaling (for the state update)
                    ktm_ps = ps_small.tile([128, 2, C], BF16, tag="small",
                                           name=f"ktm{ch}")
                    for tb in range(2):
                        for b in range(2):
                            nc.tensor.transpose(
                                ktm_ps[:, tb, b * 128:(b + 1) * 128],
                                ks[:, b, tb * 128:(tb + 1) * 128],
                                identmm)
                    kdkv = qk_pool.tile([128, 2, C], BF16, tag="kdkv")
                    nc.vector.tensor_mul(
                        kdkv, ktm_ps,
                        gkc[:, None, :].broadcast_to([128, 2, C]))

                    o_ps_pairs = [ps_o.tile([128, 2, C], F32, tag="o",
                                            name=f"o_ps{ch}_{i}")
                                  for i in range(2)]
                    a_sbs = [None] * HPC

                    def emit_at(h):
                        blk, pb = h // 2, (h % 2) * 64
                        at_ps = ps_small.tile([128, 2, C], F32, tag="small",
                                              name=f"at{ch}_{h}")
                        nc.tensor.matmul(at_ps[:, 0, :],
                                         lhsT=ks[pb:pb + 64, blk, 0:128],
                                         rhs=qs[pb:pb + 64, blk, :],
                                         start=True, stop=True)
                        nc.tensor.matmul(at_ps[:, 1, 128:256],
                                         lhsT=ks[pb:pb + 64, blk, 128:256],
                                         rhs=qs[pb:pb + 64, blk, 128:256],
                                         start=True, stop=True)
                        a_sb = a_pool.tile([128, 2, C], BF16, tag="a",
                                           name=f"a{ch}_{h}")
                        nc.vector.tensor_mul(a_sb, at_ps, jtm)
                        a_sbs[h] = a_sb

                    def emit_o(h):
                        # intra-chunk matmuls first; the o_inter matmul last
                        # so the cross-chunk state dependency lands as late
                        # as possible in the PE queue
                        p, hh = h // 2, h % 2
                        blk, pb = h // 2, (h % 2) * 64
                        o_slice = o_ps_pairs[p][:, hh, :]
                        vl = v_sb[:, vtb0, h * 128:(h + 1) * 128]
                        a_sb = a_sbs[h]
                        if ch > 0:
                            # inter first, jb1, then one merged full-width
                            # jb0 matmul carrying the single stop (last
                            # writer of both psum halves)
                            nc.tensor.matmul(
                                o_slice,
                                lhsT=s_sb[hh * 64:hh * 64 + 64, p,
                                          hh * 128:(hh + 1) * 128],
                                rhs=qs[pb:pb + 64, blk, :],
                                start=True, stop=False)
                            nc.tensor.matmul(o_slice[:, 128:256],
                                             lhsT=v_sb[:, vtb0 + 1,
                                                       h * 128:(h + 1) * 128],
                                             rhs=a_sb[:, 1, 128:256],
                                             start=False, stop=False)
                            nc.tensor.matmul(o_slice, lhsT=vl,
                                             rhs=a_sb[:, 0, :],
                                             start=False, stop=True)
                        else:
                            nc.tensor.matmul(o_slice[:, 0:128], lhsT=vl,
                                             rhs=a_sb[:, 0, 0:128],
                                             start=True, stop=True)
                            nc.tensor.matmul(o_slice[:, 128:256], lhsT=vl,
                                             rhs=a_sb[:, 0, 128:256],
                                             start=True, stop=False)
                            nc.tensor.matmul(o_slice[:, 128:256],
                                             lhsT=v_sb[:, vtb0 + 1,
                                                       h * 128:(h + 1) * 128],
                                             rhs=a_sb[:, 1, 128:256],
                                             start=False, stop=True)

                    def norm_front(p):
                        """scalar/vector front half of the norm chain."""
                        o_ps = o_ps_pairs[p]
                        o_flat = o_ps.rearrange('p a b -> p (a b)')
                        osc = nrm_pool.tile([128, 512], F32, tag="osc",
                                            name=f"osc{ch}_{p}")
                        nc.scalar.copy(osc, o_flat)
                        o2 = nrm_pool.tile([128, 512], BF16, tag="o2",
                                           name=f"o2{ch}_{p}")
                        nc.vector.tensor_mul(o2, osc, o_flat)
                        return osc, o2

                    def norm_mid(p, osc, o2):
                        """mean matmul + rsqrt + partition broadcast."""
                        mean_ps = ps_small.tile([1, 512], F32, tag="small",
                                                name=f"mean{ch}_{p}")
                        nc.tensor.matmul(mean_ps, lhsT=ones, rhs=o2,
                                         start=True, stop=True)
                        rsq1 = nrm_pool.tile([1, 512], F32, tag="rsq1",
                                             name=f"rsq{ch}_{p}")
                        nc.scalar.activation(rsq1, mean_ps,
                                             AF.Abs_reciprocal_sqrt,
                                             bias=epsb, scale=1.0 / DV)
                        rsqf = nrm_pool.tile([128, 512], F32, tag="rsqf",
                                             name=f"rsqf{ch}_{p}")
                        nc.gpsimd.partition_broadcast(rsqf, rsq1)
                        return rsqf

                    emit_at(0)
                    emit_at(1)

                    # state-update matmuls early: inputs (kdkv, v) are ready
                    # at chunk start, and the s_sb write then lands well
                    # before the next chunk's o_inter reads it
                    ds_ps = ps_small.tile([128, 2, C], F32, tag="small",
                                          name=f"ds{ch}")
                    for p in range(2):
                        for jb in range(2):
                            nc.tensor.matmul(
                                ds_ps[:, p, :],
                                lhsT=kdkv[:, jb, p * 128:(p + 1) * 128],
                                rhs=v_sb[:, vtb0 + jb, p * 256:(p + 1) * 256],
                                start=(jb == 0), stop=(jb == 1))

                    emit_o(0)
                    emit_at(2)
                    emit_o(1)            # pair 0 o_ps complete
                    nf0 = norm_front(0)
                    emit_at(3)
                    emit_o(2)
                    emit_o(HPC - 1)      # pair 1 o_ps complete

                    for p in range(2):
                        nc.vector.scalar_tensor_tensor(
                            out=s_sb[:, p, :],
                            in0=s_sb[:, p, :],
                            scalar=gcv[:, p:p + 1],
                            in1=ds_ps[:, p, :],
                            op0=mybir.AluOpType.mult,
                            op1=mybir.AluOpType.add)
                    nf1 = norm_front(1)
                    if ch == NCH - 1:
                        rsqf1 = norm_mid(1, *nf1)
                    rsqf0 = norm_mid(0, *nf0)

                    # deferred norm tail + output projection of older
                    # chunks (two-chunk deferral: og is never the limiter)
                    while pending_wo:
                        pnf0, pnf1, prs0, prs1, pgsil, pch, pcc, pc0 = \
                            pending_wo.pop(0)
                        og_prev = [norm_tail_p(0, pnf0[0], prs0, pgsil,
                                               pch, pcc),
                                   norm_tail_p(1, pnf1[0], prs1, pgsil,
                                               pch, pcc)]
                        emit_wo(og_prev, pc0)
                    if ch != NCH - 1:
                        rsqf1 = norm_mid(1, *nf1)

                    if ch == NCH - 1:
                        og_pairs = [norm_tail_p(0, nf0[0], rsqf0, gsil,
                                                ch, cc),
                                    norm_tail_p(1, nf1[0], rsqf1, gsil,
                                                ch, cc)]
                        emit_wo(og_pairs, c0, last=True)
                    else:
                        pending_wo.append((nf0, nf1, rsqf0, rsqf1, gsil,
                                           ch, cc, c0))

    nc.finalize()
    return nc


def _host_tables(heads):
    """Per-core constant tables for a 4-head slice."""
    gam = (1.0 - 2.0 ** (-5.0 - np.arange(H, dtype=np.float64)))[heads]  # [4]

    # rope tables with retention decay folded in, feature-major [128, 4, T]
    # (cos m0, cos m1, sin m0, sin m1); rows permuted by P64 within each
    # 64-dim head block so rotate partners share a 32-partition quadrant
    inv = 10000.0 ** (-np.arange(0, DK, 2, dtype=np.float64) / DK)  # [32]
    t_idx = np.arange(T, dtype=np.float64)
    ang = np.outer(t_idx, inv)                      # [T, 32]
    cos_t, sin_t = np.cos(ang), np.sin(ang)         # [T, 32]
    i_in_chunk = (np.arange(T) % C).astype(np.float64)

    CSQ = np.empty((128, 4, T), np.float64)
    CSK = np.empty((128, 4, T), np.float64)
    for m in range(2):
        for p in range(128):
            f = m * 128 + p            # feature index within the 4-head slice
            hc = f // 64               # head-local index 0..3
            dd = P64[f % 64]           # original dim within the head
            idx = dd % 32
            sign = 1.0 if dd < 32 else -1.0
            dq = gam[hc] ** (i_in_chunk + 1.0)
            dk = gam[hc] ** (-i_in_chunk - 1.0)
            CSQ[p, m, :] = cos_t[:, idx] * dq
            CSQ[p, 2 + m, :] = sign * sin_t[:, idx] * dq
            CSK[p, m, :] = cos_t[:, idx] * dk
            CSK[p, 2 + m, :] = sign * sin_t[:, idx] * dk

    # full-tile A mask [128(j), 2(jb), 256(i)]: jb0 = [tri | ones]
    # (diagonal block then unmasked off-diagonal), jb1 = [zero | tri]
    # (the never-read strictly-upper quarter stays zeroed)
    j_idx = np.arange(128)
    tri = (j_idx[:, None] <= j_idx[None, :]).astype(np.float32)
    on = np.ones((128, 128), np.float32)
    JTt = np.stack([np.concatenate([tri, on], 1),
                    np.concatenate([0 * on, tri], 1)], axis=1)

    # gamma^C per k-token-major column (column c -> head c//64)
    GKCt = np.broadcast_to(
        np.repeat(gam ** C, 64)[None, :], (128, 256)).astype(np.float32)

    # gamma^C per state-pair row
    GCVt = np.empty((128, 2), np.float32)
    for p in range(2):
        GCVt[0:64, p] = gam[2 * p] ** C
        GCVt[64:128, p] = gam[2 * p + 1] ** C
    return CSQ, CSK, JTt, GKCt, GCVt


def _prepare_inputs(x, Wq, Wk, Wv, Wg, Wo, g_norm_w):
    x = np.asarray(x, np.float32)
    Wq = np.asarray(Wq, np.float32) * (DK ** -0.5)
    Wk = np.asarray(Wk, np.float32)
    Wv = np.asarray(Wv, np.float32)
    Wg = np.asarray(Wg, np.float32)
    Wo = np.asarray(Wo, np.float32)
    gw = np.asarray(g_norm_w, np.float32)

    in_maps = []
    for core in range(NCORES):
        b = core // 4
        hg = core % 4
        heads = np.arange(4 * hg, 4 * hg + 4)
        # q/k columns with the P64 row permutation applied per head
        qk_cols = np.concatenate(
            [h * DK + np.asarray(P64) for h in heads])
        vg_cols = np.concatenate([np.arange(h * DV, (h + 1) * DV) for h in heads])

        XTc = np.ascontiguousarray(x[b].T).astype(BF)
        WQc = np.ascontiguousarray(
            Wq[:, qk_cols].reshape(8, 128, 256).transpose(1, 0, 2)).astype(BF)
        WKc = np.ascontiguousarray(
            Wk[:, qk_cols].reshape(8, 128, 256).transpose(1, 0, 2)).astype(BF)
        WVc = np.ascontiguousarray(
            Wv[:, vg_cols].reshape(8, 128, 512).transpose(1, 0, 2)).astype(BF)
        WGc = np.ascontiguousarray(
            Wg[:, vg_cols].reshape(8, 128, 512).transpose(1, 0, 2)).astype(BF)
        WOc = np.ascontiguousarray(
            (Wo[vg_cols, :] * np.tile(gw, 4)[:, None])
            .reshape(4, 128, 1024).transpose(1, 0, 2)).astype(BF)

        CSQt, CSKt, JTt, GKCt, GCVt = _host_tables(heads)

        in_maps.append({
            "XT": XTc, "WQ": WQc, "WK": WKc, "WV": WVc, "WG": WGc, "WO": WOc,
            "CSQ": CSQt.astype(BF), "CSK": CSKt.astype(BF),
            "JT": JTt.astype(BF), "GKC": GKCt,
            "GCV": GCVt,
            "ONES": np.ones((128, 1), BF),
            "IDENT": np.eye(128, dtype=BF),
            "ZS": np.zeros((128, 2, C), BF),
        })
    return in_maps


def _run(in_maps, **kw):
    if "nc" not in _cache:
        _cache["nc"] = _build_program()
    return run_bass_kernel_spmd(_cache["nc"], in_maps,
                                core_ids=list(range(NCORES)), **kw)


def kernel(x, Wq, Wk, Wv, Wg, Wo, g_norm_w):
    in_maps = _prepare_inputs(x, Wq, Wk, Wv, Wg, Wo, g_norm_w)
    res = _run(in_maps)
    out = np.zeros((B, T, D), np.float32)
    for core in range(NCORES):
        out[core // 4] += res.results[core]["OUT"].astype(np.float32)
    return out


# revision 20
# speedup vs baseline: 1.1775x; 1.0010x over previous
"""FLARetNet Trainium2 kernel: 8-core SPMD, batch x head-group sharding.

Each core handles one batch (B=2 -> 4 cores per batch) and 4 of 16 heads.
Per core: qkvg projections (fp16 matmuls), neox RoPE, RetNet chunked
retention scan (chunk=256), fused RMSNorm + swish gate, output projection
(partial sum over its heads). Host sums the 4 partials per batch.

Schedule/layout notes:
- Retention decay is folded into the RoPE tables: q tables carry
  gamma^(i+1), k tables carry gamma^(-j-1) (i,j = position in chunk).
  Off-diagonal A blocks then need no mask at all; only the two 128x128
  diagonal blocks get a 0/1 triangular mask, and the strictly-upper
  block is never computed. kdkv becomes a constant gamma^C column scale.
- RoPE rotate-half runs as a DVE stream_shuffle: the q/k feature rows
  are permuted host-side (within each head's 64 dims) so rotation
  partners sit in the same 32-partition quadrant.
- Matmuls run fp16 (full PE rate); PSUM accumulation is fp32.
- The per-chunk Wo projection is deferred by one chunk and split into
  per-og-pair passes so it never waits on the full norm/gate chain.
- The rsqrt activation table is pre-warmed with a dummy op after each
  projection tile's silus so the table load stays off the norm chain.
"""
import numpy as np
import ml_dtypes

import concourse.mybir as mybir
import concourse.tile as tile
import concourse.bacc as bacc
import concourse.bass_isa as bass_isa
from concourse.bass_utils import run_bass_kernel_spmd

F32 = mybir.dt.float32
BF16 = mybir.dt.float16
AF = mybir.ActivationFunctionType
BF = np.float16

B, T, D, H = 2, 4096, 1024, 16
DK, DV = 64, 128
C = 256            # attention chunk length (math-equivalent for any C)
PT = 512           # projection token-tile
NCH = T // C       # 16 chunks
HPC = 4            # heads per core
NCORES = 8

# rope-partner shuffle: within each 32-partition quadrant swap halves
SHUF = list(range(16, 32)) + list(range(16))
# row permutation within each 64-dim head block so partners share a quadrant
P64 = list(range(16)) + list(range(32, 48)) + list(range(16, 32)) + list(range(48, 64))

_cache = {}


def _build_program():
    nc = bacc.Bacc("TRN2", target_bir_lowering=False, debug=False)

    XT = nc.dram_tensor("XT", [D, T], BF16, kind="ExternalInput")
    WQ = nc.dram_tensor("WQ", [128, 8, 256], BF16, kind="ExternalInput")
    WK = nc.dram_tensor("WK", [128, 8, 256], BF16, kind="ExternalInput")
    WV = nc.dram_tensor("WV", [128, 8, 512], BF16, kind="ExternalInput")
    WG = nc.dram_tensor("WG", [128, 8, 512], BF16, kind="ExternalInput")
    WO = nc.dram_tensor("WO", [128, 4, 1024], BF16, kind="ExternalInput")
    CSQ = nc.dram_tensor("CSQ", [128, 4, T], BF16, kind="ExternalInput")
    CSK = nc.dram_tensor("CSK", [128, 4, T], BF16, kind="ExternalInput")
    JT = nc.dram_tensor("JT", [128, 2, 256], BF16, kind="ExternalInput")
    GKC = nc.dram_tensor("GKC", [128, 256], F32, kind="ExternalInput")
    GCV = nc.dram_tensor("GCV", [128, 2], F32, kind="ExternalInput")
    IDENT = nc.dram_tensor("IDENT", [128, 128], BF16, kind="ExternalInput")
    ONES = nc.dram_tensor("ONES", [128, 1], BF16, kind="ExternalInput")
    ZS = nc.dram_tensor("ZS", [128, 2, C], BF16, kind="ExternalInput")

    OUT = nc.dram_tensor("OUT", [T, D], BF16, kind="ExternalOutput")

    with tile.TileContext(nc) as tc:
        with tc.tile_pool(name="singles", bufs=1) as singles, \
             tc.tile_pool(name="xt", bufs=2) as xt_pool, \
             tc.tile_pool(name="tab", bufs=2) as tab_pool, \
             tc.tile_pool(name="rope", bufs=2) as rope_pool, \
             tc.tile_pool(name="qk", bufs=2) as qk_pool, \
             tc.tile_pool(name="vsb", bufs=2) as v_pool, \
             tc.tile_pool(name="asb", bufs=3) as a_pool, \
             tc.tile_pool(name="gat", bufs=2) as g_pool, \
             tc.tile_pool(name="nrm", bufs=4) as nrm_pool, \
             tc.tile_pool(name="og", bufs=4) as og_pool, \
             tc.tile_pool(name="osb", bufs=3) as out_pool, \
             tc.tile_pool(name="ps_proj", bufs=2, space="PSUM") as ps_proj, \
             tc.tile_pool(name="ps_small", bufs=3, space="PSUM") as ps_small, \
             tc.tile_pool(name="ps_o", bufs=3, space="PSUM") as ps_o:

            # ---- resident weights/tables (usage order; wo last) ----
            wq = singles.tile([128, 8, 256], BF16)
            wk = singles.tile([128, 8, 256], BF16)
            wv = singles.tile([128, 8, 512], BF16)
            wg = singles.tile([128, 8, 512], BF16)
            wo = singles.tile([128, 4, 1024], BF16)
            identmm = singles.tile([128, 128], BF16)
            jtm = singles.tile([128, 2, 256], BF16)
            gkc = singles.tile([128, 256], F32)
            gcv = singles.tile([128, 2], F32)
            ones = singles.tile([128, 1], BF16)
            s_sb = singles.tile([128, 2, C], BF16)

            nc.sync.dma_start(out=wq[:, :, 0:128], in_=WQ[:, :, 0:128])
            nc.sync.dma_start(out=wq[:, :, 128:256], in_=WQ[:, :, 128:256])
            nc.gpsimd.dma_start(out=wk, in_=WK[:, :, :])
            nc.gpsimd.dma_start(out=wv, in_=WV[:, :, :])
            nc.gpsimd.dma_start(out=wg, in_=WG[:, :, :])
            nc.gpsimd.dma_start(out=identmm, in_=IDENT[:, :])
            nc.gpsimd.dma_start(out=jtm, in_=JT[:, :, :])
            nc.gpsimd.dma_start(out=gkc, in_=GKC[:, :])
            nc.gpsimd.dma_start(out=gcv, in_=GCV[:, :])
            nc.gpsimd.dma_start(out=ones, in_=ONES[:, :])
            nc.gpsimd.dma_start(out=s_sb, in_=ZS[:, :, :])
            nc.gpsimd.dma_start(out=wo, in_=WO[:, :, :])

            epsb = singles.tile([1, 1], F32)
            nc.vector.memset(epsb, 1e-5)

            def emit_wo(og_pair_list, oc0, last=False):
                for tb in range(2):
                    out_ps = [ps_small.tile([128, 512], F32, tag="small",
                                            name=f"wo{oc0}_{tb}_{nn}")
                              for nn in range(2)]
                    # pair-0 heads first so this can start before pair-1's
                    # norm chain finishes
                    for hh in range(2):
                        for nn in range(2):
                            nc.tensor.matmul(
                                out_ps[nn],
                                lhsT=og_pair_list[0][
                                    :, hh, tb * 128:(tb + 1) * 128],
                                rhs=wo[:, hh, nn * 512:(nn + 1) * 512],
                                start=(hh == 0), stop=False)
                    for hh in range(2):
                        for nn in range(2):
                            nc.tensor.matmul(
                                out_ps[nn],
                                lhsT=og_pair_list[1][
                                    :, hh, tb * 128:(tb + 1) * 128],
                                rhs=wo[:, 2 + hh, nn * 512:(nn + 1) * 512],
                                start=False, stop=(hh == 1))
                    for nn in range(2):
                        out_sb = out_pool.tile([128, 512], BF16, tag="outsb",
                                               name=f"wos{oc0}_{tb}_{nn}")
                        if last and nn == 1:
                            nc.vector.tensor_copy(out=out_sb, in_=out_ps[nn])
                        else:
                            nc.scalar.copy(out_sb, out_ps[nn])
                        nc.sync.dma_start(
                            out=OUT[oc0 + tb * 128:oc0 + (tb + 1) * 128,
                                    nn * 512:(nn + 1) * 512],
                            in_=out_sb)

            pending_wo = []

            def norm_tail_p(p, osc, rsqf, gsil_t, cch, ccc):
                onrm = nrm_pool.tile([128, 512], F32, tag="onrm",
                                     name=f"onrm{cch}_{p}")
                nc.vector.tensor_mul(onrm, osc, rsqf)
                og = og_pool.tile([128, 2, C], BF16, tag="og",
                                  name=f"og{cch}_{p}")
                gs = gsil_t[:, p * 2:(p + 1) * 2, ccc * C:(ccc + 1) * C]
                nc.vector.tensor_mul(
                    og, onrm.rearrange('p (a b) -> p a b', a=2), gs)
                return og

            for pt in range(T // PT):
                p0 = pt * PT

                xt = xt_pool.tile([128, 8, PT], BF16, tag="xt")
                xt_src = XT.rearrange("(db p) t -> p db t", p=128)
                for xq in range(4):
                    eng = nc.sync if xq % 2 == 0 else nc.scalar
                    eng.dma_start(out=xt[:, 2 * xq:2 * xq + 2, :],
                                  in_=xt_src[:, 2 * xq:2 * xq + 2,
                                             p0:p0 + PT])

                csq = tab_pool.tile([128, 4, PT], BF16, tag="csq")
                nc.sync.dma_start(out=csq, in_=CSQ[:, :, p0:p0 + PT])
                csk = tab_pool.tile([128, 4, PT], BF16, tag="csk")
                nc.scalar.dma_start(out=csk, in_=CSK[:, :, p0:p0 + PT])

                # ---- projections over PT tokens ----
                # q, k feature-major [128(dim%128), blk, tok]; RoPE fused and
                # the retention decay folded into the cos/sin tables
                def proj_rope(w, cs, tag):
                    out = qk_pool.tile([128, 2, PT], BF16, tag=tag,
                                       name=f"{tag}{pt}")
                    for m in range(2):
                        pps = ps_proj.tile([128, PT], F32, tag="proj",
                                           name=f"{tag}ps{pt}_{m}")
                        for db in range(8):
                            nc.tensor.matmul(
                                pps, lhsT=w[:, db, m * 128:(m + 1) * 128],
                                rhs=xt[:, db, :],
                                start=(db == 0), stop=(db == 7))
                        tcos = rope_pool.tile([128, PT], BF16, tag="tcos")
                        tsin = rope_pool.tile([128, PT], BF16, tag="tsin")
                        rot = rope_pool.tile([128, PT], BF16, tag="rot")
                        nc.vector.tensor_mul(tcos, pps, cs[:, m, :])
                        nc.vector.tensor_mul(tsin, pps, cs[:, 2 + m, :])
                        nc.vector.stream_shuffle(rot, tsin, SHUF)
                        nc.vector.tensor_add(out[:, m, :], tcos, rot)
                    return out

                q_sb = proj_rope(wq, csq, "q")   # gamma^(i+1) q, fp16
                k_sb = proj_rope(wk, csk, "k")   # gamma^(-j-1) k, fp16

                # v token-major [128(tok%128), tb, dim]
                v_sb = v_pool.tile([128, 4, 512], BF16, tag="v")
                for tb in range(4):
                    v_ps = ps_proj.tile([128, 512], F32, tag="proj",
                                        name=f"vps{pt}_{tb}")
                    for db in range(8):
                        nc.tensor.matmul(
                            v_ps,
                            lhsT=xt[:, db, tb * 128:(tb + 1) * 128],
                            rhs=wv[:, db, :],
                            start=(db == 0), stop=(db == 7))
                    nc.scalar.copy(v_sb[:, tb, :], v_ps)

                # g feature-major per head-block -> silu
                gsil = g_pool.tile([128, 4, PT], BF16, tag="gsil")
                for m in range(4):
                    g_ps = ps_proj.tile([128, PT], F32, tag="proj",
                                        name=f"gps{pt}_{m}")
                    for db in range(8):
                        nc.tensor.matmul(
                            g_ps, lhsT=wg[:, db, m * 128:(m + 1) * 128],
                            rhs=xt[:, db, :],
                            start=(db == 0), stop=(db == 7))
                    nc.scalar.activation(gsil[:, m, :], g_ps, AF.Silu)

                # ---- per 256-chunk attention ----
                for cc in range(PT // C):
                    ch = pt * (PT // C) + cc
                    c0 = ch * C
                    qs = q_sb[:, :, cc * C:(cc + 1) * C]
                    ks = k_sb[:, :, cc * C:(cc + 1) * C]
                    vtb0 = cc * 2

                    # k token-major + gamma^C scaling (for the state update)
                    ktm_ps = ps_small.tile([128, 2, C], BF16, tag="small",
                                           name=f"ktm{ch}")
                    for tb in range(2):
                        for b in range(2):
                            nc.tensor.transpose(
                                ktm_ps[:, tb, b * 128:(b + 1) * 128],
                                ks[:, b, tb * 128:(tb + 1) * 128],
                                identmm)
                    kdkv = qk_pool.tile([128, 2, C], BF16, tag="kdkv")
                    nc.vector.tensor_mul(
                        kdkv, ktm_ps,
                        gkc[:, None, :].broadcast_to([128, 2, C]))

                    o_ps_pairs = [ps_o.tile([128, 2, C], F32, tag="o",
                                            name=f"o_ps{ch}_{i}")
                                  for i in range(2)]
                    a_sbs = [None] * HPC

                    def emit_at(h):
                        blk, pb = h // 2, (h % 2) * 64
                        at_ps = ps_small.tile([128, 2, C], F32, tag="small",
                                              name=f"at{ch}_{h}")
                        nc.tensor.matmul(at_ps[:, 0, :],
                                         lhsT=ks[pb:pb + 64, blk, 0:128],
                                         rhs=qs[pb:pb + 64, blk, :],
                                         start=True, stop=True)
                        nc.tensor.matmul(at_ps[:, 1, 128:256],
                                         lhsT=ks[pb:pb + 64, blk, 128:256],
                                         rhs=qs[pb:pb + 64, blk, 128:256],
                                         start=True, stop=True)
                        a_sb = a_pool.tile([128, 2, C], BF16, tag="a",
                                           name=f"a{ch}_{h}")
                        nc.vector.tensor_mul(a_sb, at_ps, jtm)
                        a_sbs[h] = a_sb

                    def emit_o(h):
                        # intra-chunk matmuls first; the o_inter matmul last
                        # so the cross-chunk state dependency lands as late
                        # as possible in the PE queue
                        p, hh = h // 2, h % 2
                        blk, pb = h // 2, (h % 2) * 64
                        o_slice = o_ps_pairs[p][:, hh, :]
                        vl = v_sb[:, vtb0, h * 128:(h + 1) * 128]
                        a_sb = a_sbs[h]
                        if ch > 0:
                            # inter first, jb1, then one merged full-width
                            # jb0 matmul carrying the single stop (last
                            # writer of both psum halves)
                            nc.tensor.matmul(
                                o_slice,
                                lhsT=s_sb[hh * 64:hh * 64 + 64, p,
                                          hh * 128:(hh + 1) * 128],
                                rhs=qs[pb:pb + 64, blk, :],
                                start=True, stop=False)
                            nc.tensor.matmul(o_slice[:, 128:256],
                                             lhsT=v_sb[:, vtb0 + 1,
                                                       h * 128:(h + 1) * 128],
                                             rhs=a_sb[:, 1, 128:256],
                                             start=False, stop=False)
                            nc.tensor.matmul(o_slice, lhsT=vl,
                                             rhs=a_sb[:, 0, :],
                                             start=False, stop=True)
                        else:
                            nc.tensor.matmul(o_slice[:, 0:128], lhsT=vl,
                                             rhs=a_sb[:, 0, 0:128],
                                             start=True, stop=True)
                            nc.tensor.matmul(o_slice[:, 128:256], lhsT=vl,
                                             rhs=a_sb[:, 0, 128:256],
                                             start=True, stop=False)
                            nc.tensor.matmul(o_slice[:, 128:256],
                                             lhsT=v_sb[:, vtb0 + 1,
                                                       h * 128:(h + 1) * 128],
                                             rhs=a_sb[:, 1, 128:256],
                                             start=False, stop=True)

                    def norm_front(p):
                        """scalar/vector front half of the norm chain."""
                        o_ps = o_ps_pairs[p]
                        o_flat = o_ps.rearrange('p a b -> p (a b)')
                        osc = nrm_pool.tile([128, 512], F32, tag="osc",
                                            name=f"osc{ch}_{p}")
                        nc.scalar.copy(osc, o_flat)
                        o2 = nrm_pool.tile([128, 512], BF16, tag="o2",
                                           name=f"o2{ch}_{p}")
                        nc.vector.tensor_mul(o2, osc, o_flat)
                        return osc, o2

                    def norm_mid(p, osc, o2):
                        """mean matmul + rsqrt + partition broadcast."""
                        mean_ps = ps_small.tile([1, 512], F32, tag="small",
                                                name=f"mean{ch}_{p}")
                        nc.tensor.matmul(mean_ps, lhsT=ones, rhs=o2,
                                         start=True, stop=True)
                        rsq1 = nrm_pool.tile([1, 512], F32, tag="rsq1",
                                             name=f"rsq{ch}_{p}")
                        nc.scalar.activation(rsq1, mean_ps,
                                             AF.Abs_reciprocal_sqrt,
                                             bias=epsb, scale=1.0 / DV)
                        rsqf = nrm_pool.tile([128, 512], F32, tag="rsqf",
                                             name=f"rsqf{ch}_{p}")
                        nc.gpsimd.partition_broadcast(rsqf, rsq1)
                        return rsqf

                    emit_at(0)
                    emit_at(1)

                    # state-update matmuls early: inputs (kdkv, v) are ready
                    # at chunk start, and the s_sb write then lands well
                    # before the next chunk's o_inter reads it
                    ds_ps = ps_small.tile([128, 2, C], F32, tag="small",
                                          name=f"ds{ch}")
                    for p in range(2):
                        for jb in range(2):
                            nc.tensor.matmul(
                                ds_ps[:, p, :],
                                lhsT=kdkv[:, jb, p * 128:(p + 1) * 128],
                                rhs=v_sb[:, vtb0 + jb, p * 256:(p + 1) * 256],
                                start=(jb == 0), stop=(jb == 1))

                    emit_o(0)
                    emit_at(2)
                    emit_o(1)            # pair 0 o_ps complete
                    nf0 = norm_front(0)
                    emit_at(3)
                    emit_o(2)
                    emit_o(HPC - 1)      # pair 1 o_ps complete

                    for p in range(2):
                        nc.vector.scalar_tensor_tensor(
                            out=s_sb[:, p, :],
                            in0=s_sb[:, p, :],
                            scalar=gcv[:, p:p + 1],
                            in1=ds_ps[:, p, :],
                            op0=mybir.AluOpType.mult,
                            op1=mybir.AluOpType.add)
                    nf1 = norm_front(1)
                    if ch == NCH - 1:
                        rsqf1 = norm_mid(1, *nf1)
                    rsqf0 = norm_mid(0, *nf0)

                    # deferred norm tail + output projection of older
                    # chunks (two-chunk deferral: og is never the limiter)
                    while pending_wo:
                        pnf0, pnf1, prs0, prs1, pgsil, pch, pcc, pc0 = \
                            pending_wo.pop(0)
                        og_prev = [norm_tail_p(0, pnf0[0], prs0, pgsil,
                                               pch, pcc),
                                   norm_tail_p(1, pnf1[0], prs1, pgsil,
                                               pch, pcc)]
                        emit_wo(og_prev, pc0)
                    if ch != NCH - 1:
                        rsqf1 = norm_mid(1, *nf1)

                    if ch == NCH - 1:
                        og_pairs = [norm_tail_p(0, nf0[0], rsqf0, gsil,
                                                ch, cc),
                                    norm_tail_p(1, nf1[0], rsqf1, gsil,
                                                ch, cc)]
                        emit_wo(og_pairs, c0, last=True)
                    else:
                        pending_wo.append((nf0, nf1, rsqf0, rsqf1, gsil,
                                           ch, cc, c0))

    nc.finalize()
    return nc


def _host_tables(heads):
    """Per-core constant tables for a 4-head slice."""
    gam = (1.0 - 2.0 ** (-5.0 - np.arange(H, dtype=np.float64)))[heads]  # [4]

    # rope tables with retention decay folded in, feature-major [128, 4, T]
    # (cos m0, cos m1, sin m0, sin m1); rows permuted by P64 within each
    # 64-dim head block so rotate partners share a 32-partition quadrant
    inv = 10000.0 ** (-np.arange(0, DK, 2, dtype=np.float64) / DK)  # [32]
    t_idx = np.arange(T, dtype=np.float64)
    ang = np.outer(t_idx, inv)                      # [T, 32]
    cos_t, sin_t = np.cos(ang), np.sin(ang)         # [T, 32]
    i_in_chunk = (np.arange(T) % C).astype(np.float64)

    CSQ = np.empty((128, 4, T), np.float64)
    CSK = np.empty((128, 4, T), np.float64)
    for m in range(2):
        for p in range(128):
            f = m * 128 + p            # feature index within the 4-head slice
            hc = f // 64               # head-local index 0..3
            dd = P64[f % 64]           # original dim within the head
            idx = dd % 32
            sign = 1.0 if dd < 32 else -1.0
            dq = gam[hc] ** (i_in_chunk + 1.0)
            dk = gam[hc] ** (-i_in_chunk - 1.0)
            CSQ[p, m, :] = cos_t[:, idx] * dq
            CSQ[p, 2 + m, :] = sign * sin_t[:, idx] * dq
            CSK[p, m, :] = cos_t[:, idx] * dk
            CSK[p, 2 + m, :] = sign * sin_t[:, idx] * dk

    # full-tile A mask [128(j), 2(jb), 256(i)]: jb0 = [tri | ones]
    # (diagonal block then unmasked off-diagonal), jb1 = [zero | tri]
    # (the never-read strictly-upper quarter stays zeroed)
    j_idx = np.arange(128)
    tri = (j_idx[:, None] <= j_idx[None, :]).astype(np.float32)
    on = np.ones((128, 128), np.float32)
    JTt = np.stack([np.concatenate([tri, on], 1),
                    np.concatenate([0 * on, tri], 1)], axis=1)

    # gamma^C per k-token-major column (column c -> head c//64)
    GKCt = np.broadcast_to(
        np.repeat(gam ** C, 64)[None, :], (128, 256)).astype(np.float32)

    # gamma^C per state-pair row
    GCVt = np.empty((128, 2), np.float32)
    for p in range(2):
        GCVt[0:64, p] = gam[2 * p] ** C
        GCVt[64:128, p] = gam[2 * p + 1] ** C
    return CSQ, CSK, JTt, GKCt, GCVt


def _prepare_inputs(x, Wq, Wk, Wv, Wg, Wo, g_norm_w):
    x = np.asarray(x, np.float32)
    Wq = np.asarray(Wq, np.float32) * (DK ** -0.5)
    Wk = np.asarray(Wk, np.float32)
    Wv = np.asarray(Wv, np.float32)
    Wg = np.asarray(Wg, np.float32)
    Wo = np.asarray(Wo, np.float32)
    gw = np.asarray(g_norm_w, np.float32)

    in_maps = []
    for core in range(NCORES):
        b = core // 4
        hg = core % 4
        heads = np.arange(4 * hg, 4 * hg + 4)
        # q/k columns with the P64 row permutation applied per head
        qk_cols = np.concatenate(
            [h * DK + np.asarray(P64) for h in heads])
        vg_cols = np.concatenate([np.arange(h * DV, (h + 1) * DV) for h in heads])

        XTc = np.ascontiguousarray(x[b].T).astype(BF)
        WQc = np.ascontiguousarray(
            Wq[:, qk_cols].reshape(8, 128, 256).transpose(1, 0, 2)).astype(BF)
        WKc = np.ascontiguousarray(
            Wk[:, qk_cols].reshape(8, 128, 256).transpose(1, 0, 2)).astype(BF)
        WVc = np.ascontiguousarray(
            Wv[:, vg_cols].reshape(8, 128, 512).transpose(1, 0, 2)).astype(BF)
        WGc = np.ascontiguousarray(
            Wg[:, vg_cols].reshape(8, 128, 512).transpose(1, 0, 2)).astype(BF)
        WOc = np.ascontiguousarray(
            (Wo[vg_cols, :] * np.tile(gw, 4)[:, None])
            .reshape(4, 128, 1024).transpose(1, 0, 2)).astype(BF)

        CSQt, CSKt, JTt, GKCt, GCVt = _host_tables(heads)

        in_maps.append({
            "XT": XTc, "WQ": WQc, "WK": WKc, "WV": WVc, "WG": WGc, "WO": WOc,
            "CSQ": CSQt.astype(BF), "CSK": CSKt.astype(BF),
            "JT": JTt.astype(BF), "GKC": GKCt,
            "GCV": GCVt,
            "ONES": np.ones((128, 1), BF),
            "IDENT": np.eye(128, dtype=BF),
            "ZS": np.zeros((128, 2, C), BF),
        })
    return in_maps


def _run(in_maps, **kw):
    if "nc" not in _cache:
        _cache["nc"] = _build_program()
    return run_bass_kernel_spmd(_cache["nc"], in_maps,
                                core_ids=list(range(NCORES)), **kw)


def kernel(x, Wq, Wk, Wv, Wg, Wo, g_norm_w):
    in_maps = _prepare_inputs(x, Wq, Wk, Wv, Wg, Wo, g_norm_w)
    res = _run(in_maps)
    out = np.zeros((B, T, D), np.float32)
    for core in range(NCORES):
        out[core // 4] += res.results[core]["OUT"].astype(np.float32)
    return out


# revision 21
# speedup vs baseline: 1.1921x; 1.0124x over previous
"""FLARetNet Trainium2 kernel: 8-core SPMD, batch x head-group sharding.

Each core handles one batch (B=2 -> 4 cores per batch) and 4 of 16 heads.
Per core: qkvg projections (fp16 matmuls), neox RoPE, RetNet chunked
retention scan (chunk=256), fused RMSNorm + swish gate, output projection
(partial sum over its heads). Host sums the 4 partials per batch.

Schedule/layout notes:
- Retention decay is folded into the RoPE tables: q tables carry
  gamma^(i+1), k tables carry gamma^(-j-1) (i,j = position in chunk).
  Off-diagonal A blocks then need no mask at all; only the two 128x128
  diagonal blocks get a 0/1 triangular mask, and the strictly-upper
  block is never computed. kdkv becomes a constant gamma^C column scale.
- RoPE rotate-half runs as a DVE stream_shuffle: the q/k feature rows
  are permuted host-side (within each head's 64 dims) so rotation
  partners sit in the same 32-partition quadrant.
- Matmuls run fp16 (full PE rate); PSUM accumulation is fp32.
- The per-chunk Wo projection is deferred by one chunk and split into
  per-og-pair passes so it never waits on the full norm/gate chain.
- The rsqrt activation table is pre-warmed with a dummy op after each
  projection tile's silus so the table load stays off the norm chain.
"""
import numpy as np
import ml_dtypes

import concourse.mybir as mybir
import concourse.tile as tile
import concourse.bacc as bacc
import concourse.bass_isa as bass_isa
from concourse.bass_utils import run_bass_kernel_spmd

F32 = mybir.dt.float32
BF16 = mybir.dt.float16
AF = mybir.ActivationFunctionType
BF = np.float16

B, T, D, H = 2, 4096, 1024, 16
DK, DV = 64, 128
C = 256            # attention chunk length (math-equivalent for any C)
PT = 512           # projection token-tile
NCH = T // C       # 16 chunks
HPC = 4            # heads per core
NCORES = 8

# rope-partner shuffle: within each 32-partition quadrant swap halves
SHUF = list(range(16, 32)) + list(range(16))
# row permutation within each 64-dim head block so partners share a quadrant
P64 = list(range(16)) + list(range(32, 48)) + list(range(16, 32)) + list(range(48, 64))

_cache = {}


def _build_program():
    nc = bacc.Bacc("TRN2", target_bir_lowering=False, debug=False)

    XT = nc.dram_tensor("XT", [D, T], BF16, kind="ExternalInput")
    WQ = nc.dram_tensor("WQ", [128, 8, 256], BF16, kind="ExternalInput")
    WK = nc.dram_tensor("WK", [128, 8, 256], BF16, kind="ExternalInput")
    WV = nc.dram_tensor("WV", [128, 8, 512], BF16, kind="ExternalInput")
    WG = nc.dram_tensor("WG", [128, 8, 512], BF16, kind="ExternalInput")
    WO = nc.dram_tensor("WO", [128, 4, 1024], BF16, kind="ExternalInput")
    CSQ = nc.dram_tensor("CSQ", [128, 4, T], BF16, kind="ExternalInput")
    CSK = nc.dram_tensor("CSK", [128, 4, T], BF16, kind="ExternalInput")
    JT = nc.dram_tensor("JT", [128, 2, 256], BF16, kind="ExternalInput")
    GKC = nc.dram_tensor("GKC", [128, 256], F32, kind="ExternalInput")
    GCV = nc.dram_tensor("GCV", [128, 2], F32, kind="ExternalInput")
    IDENT = nc.dram_tensor("IDENT", [128, 128], BF16, kind="ExternalInput")
    ONES = nc.dram_tensor("ONES", [128, 1], BF16, kind="ExternalInput")
    ZS = nc.dram_tensor("ZS", [128, 2, C], BF16, kind="ExternalInput")

    OUT = nc.dram_tensor("OUT", [T, D], BF16, kind="ExternalOutput")

    with tile.TileContext(nc) as tc:
        with tc.tile_pool(name="singles", bufs=1) as singles, \
             tc.tile_pool(name="xt", bufs=2) as xt_pool, \
             tc.tile_pool(name="tab", bufs=2) as tab_pool, \
             tc.tile_pool(name="rope", bufs=2) as rope_pool, \
             tc.tile_pool(name="qk", bufs=2) as qk_pool, \
             tc.tile_pool(name="vsb", bufs=2) as v_pool, \
             tc.tile_pool(name="asb", bufs=3) as a_pool, \
             tc.tile_pool(name="gat", bufs=2) as g_pool, \
             tc.tile_pool(name="nrm", bufs=4) as nrm_pool, \
             tc.tile_pool(name="og", bufs=4) as og_pool, \
             tc.tile_pool(name="osb", bufs=3) as out_pool, \
             tc.tile_pool(name="ps_proj", bufs=2, space="PSUM") as ps_proj, \
             tc.tile_pool(name="ps_small", bufs=3, space="PSUM") as ps_small, \
             tc.tile_pool(name="ps_o", bufs=3, space="PSUM") as ps_o:

            # ---- resident weights/tables (usage order; wo last) ----
            wq = singles.tile([128, 8, 256], BF16)
            wk = singles.tile([128, 8, 256], BF16)
            wv = singles.tile([128, 8, 512], BF16)
            wg = singles.tile([128, 8, 512], BF16)
            wo = singles.tile([128, 4, 1024], BF16)
            identmm = singles.tile([128, 128], BF16)
            jtm = singles.tile([128, 2, 256], BF16)
            gkc = singles.tile([128, 256], F32)
            gcv = singles.tile([128, 2], F32)
            ones = singles.tile([128, 1], BF16)
            s_sb = singles.tile([128, 2, C], BF16)

            nc.gpsimd.dma_start(out=wq, in_=WQ[:, :, :])
            nc.gpsimd.dma_start(out=wk, in_=WK[:, :, :])
            nc.gpsimd.dma_start(out=wv, in_=WV[:, :, :])
            nc.gpsimd.dma_start(out=wg, in_=WG[:, :, :])
            nc.gpsimd.dma_start(out=identmm, in_=IDENT[:, :])
            nc.gpsimd.dma_start(out=jtm, in_=JT[:, :, :])
            nc.gpsimd.dma_start(out=gkc, in_=GKC[:, :])
            nc.gpsimd.dma_start(out=gcv, in_=GCV[:, :])
            nc.gpsimd.dma_start(out=ones, in_=ONES[:, :])
            nc.gpsimd.dma_start(out=s_sb, in_=ZS[:, :, :])
            nc.gpsimd.dma_start(out=wo, in_=WO[:, :, :])

            epsb = singles.tile([1, 1], F32)
            nc.vector.memset(epsb, 1e-5)

            def emit_wo(og_pair_list, oc0, last=False):
                for tb in range(2):
                    out_ps = [ps_small.tile([128, 512], F32, tag="small",
                                            name=f"wo{oc0}_{tb}_{nn}")
                              for nn in range(2)]
                    # pair-0 heads first so this can start before pair-1's
                    # norm chain finishes
                    for hh in range(2):
                        for nn in range(2):
                            nc.tensor.matmul(
                                out_ps[nn],
                                lhsT=og_pair_list[0][
                                    :, hh, tb * 128:(tb + 1) * 128],
                                rhs=wo[:, hh, nn * 512:(nn + 1) * 512],
                                start=(hh == 0), stop=False)
                    for hh in range(2):
                        for nn in range(2):
                            nc.tensor.matmul(
                                out_ps[nn],
                                lhsT=og_pair_list[1][
                                    :, hh, tb * 128:(tb + 1) * 128],
                                rhs=wo[:, 2 + hh, nn * 512:(nn + 1) * 512],
                                start=False, stop=(hh == 1))
                    for nn in range(2):
                        out_sb = out_pool.tile([128, 512], BF16, tag="outsb",
                                               name=f"wos{oc0}_{tb}_{nn}")
                        if last and nn == 1:
                            nc.vector.tensor_copy(out=out_sb, in_=out_ps[nn])
                        else:
                            nc.scalar.copy(out_sb, out_ps[nn])
                        nc.sync.dma_start(
                            out=OUT[oc0 + tb * 128:oc0 + (tb + 1) * 128,
                                    nn * 512:(nn + 1) * 512],
                            in_=out_sb)

            pending_wo = []

            def norm_tail_p(p, osc, rsqf, gsil_t, cch, ccc):
                onrm = nrm_pool.tile([128, 512], F32, tag="onrm",
                                     name=f"onrm{cch}_{p}")
                nc.vector.tensor_mul(onrm, osc, rsqf)
                og = og_pool.tile([128, 2, C], BF16, tag="og",
                                  name=f"og{cch}_{p}")
                gs = gsil_t[:, p * 2:(p + 1) * 2, ccc * C:(ccc + 1) * C]
                nc.vector.tensor_mul(
                    og, onrm.rearrange('p (a b) -> p a b', a=2), gs)
                return og

            for pt in range(T // PT):
                p0 = pt * PT

                xt = xt_pool.tile([128, 8, PT], BF16, tag="xt")
                xt_src = XT.rearrange("(db p) t -> p db t", p=128)
                for xq in range(4):
                    eng = nc.sync if xq % 2 == 0 else nc.scalar
                    eng.dma_start(out=xt[:, 2 * xq:2 * xq + 2, :],
                                  in_=xt_src[:, 2 * xq:2 * xq + 2,
                                             p0:p0 + PT])

                csq = tab_pool.tile([128, 4, PT], BF16, tag="csq")
                nc.sync.dma_start(out=csq, in_=CSQ[:, :, p0:p0 + PT])
                csk = tab_pool.tile([128, 4, PT], BF16, tag="csk")
                nc.scalar.dma_start(out=csk, in_=CSK[:, :, p0:p0 + PT])

                # ---- projections over PT tokens ----
                # q, k feature-major [128(dim%128), blk, tok]; RoPE fused and
                # the retention decay folded into the cos/sin tables
                def proj_rope(w, cs, tag):
                    out = qk_pool.tile([128, 2, PT], BF16, tag=tag,
                                       name=f"{tag}{pt}")
                    for m in range(2):
                        pps = ps_proj.tile([128, PT], F32, tag="proj",
                                           name=f"{tag}ps{pt}_{m}")
                        for db in range(8):
                            nc.tensor.matmul(
                                pps, lhsT=w[:, db, m * 128:(m + 1) * 128],
                                rhs=xt[:, db, :],
                                start=(db == 0), stop=(db == 7))
                        tcos = rope_pool.tile([128, PT], BF16, tag="tcos")
                        tsin = rope_pool.tile([128, PT], BF16, tag="tsin")
                        rot = rope_pool.tile([128, PT], BF16, tag="rot")
                        nc.vector.tensor_mul(tcos, pps, cs[:, m, :])
                        nc.vector.tensor_mul(tsin, pps, cs[:, 2 + m, :])
                        nc.vector.stream_shuffle(rot, tsin, SHUF)
                        nc.vector.tensor_add(out[:, m, :], tcos, rot)
                    return out

                q_sb = proj_rope(wq, csq, "q")   # gamma^(i+1) q, fp16
                k_sb = proj_rope(wk, csk, "k")   # gamma^(-j-1) k, fp16

                # v token-major [128(tok%128), tb, dim]
                v_sb = v_pool.tile([128, 4, 512], BF16, tag="v")
                for tb in range(4):
                    v_ps = ps_proj.tile([128, 512], F32, tag="proj",
                                        name=f"vps{pt}_{tb}")
                    for db in range(8):
                        nc.tensor.matmul(
                            v_ps,
                            lhsT=xt[:, db, tb * 128:(tb + 1) * 128],
                            rhs=wv[:, db, :],
                            start=(db == 0), stop=(db == 7))
                    nc.scalar.copy(v_sb[:, tb, :], v_ps)

                # g feature-major per head-block -> silu
                gsil = g_pool.tile([128, 4, PT], BF16, tag="gsil")
                for m in range(4):
                    g_ps = ps_proj.tile([128, PT], F32, tag="proj",
                                        name=f"gps{pt}_{m}")
                    for db in range(8):
                        nc.tensor.matmul(
                            g_ps, lhsT=wg[:, db, m * 128:(m + 1) * 128],
                            rhs=xt[:, db, :],
                            start=(db == 0), stop=(db == 7))
                    nc.scalar.activation(gsil[:, m, :], g_ps, AF.Silu)

                # ---- per 256-chunk attention ----
                for cc in range(PT // C):
                    ch = pt * (PT // C) + cc
                    c0 = ch * C
                    qs = q_sb[:, :, cc * C:(cc + 1) * C]
                    ks = k_sb[:, :, cc * C:(cc + 1) * C]
                    vtb0 = cc * 2

                    # k token-major + gamma^C scaling (for the state update)
                    ktm_ps = ps_small.tile([128, 2, C], BF16, tag="small",
                                           name=f"ktm{ch}")
                    for tb in range(2):
                        for b in range(2):
                            nc.tensor.transpose(
                                ktm_ps[:, tb, b * 128:(b + 1) * 128],
                                ks[:, b, tb * 128:(tb + 1) * 128],
                                identmm)
                    kdkv = qk_pool.tile([128, 2, C], BF16, tag="kdkv")
                    nc.vector.tensor_mul(
                        kdkv, ktm_ps,
                        gkc[:, None, :].broadcast_to([128, 2, C]))

                    o_ps_pairs = [ps_o.tile([128, 2, C], F32, tag="o",
                                            name=f"o_ps{ch}_{i}")
                                  for i in range(2)]
                    a_sbs = [None] * HPC

                    def emit_at(h):
                        blk, pb = h // 2, (h % 2) * 64
                        at_ps = ps_small.tile([128, 2, C], F32, tag="small",
                                              name=f"at{ch}_{h}")
                        nc.tensor.matmul(at_ps[:, 0, :],
                                         lhsT=ks[pb:pb + 64, blk, 0:128],
                                         rhs=qs[pb:pb + 64, blk, :],
                                         start=True, stop=True)
                        nc.tensor.matmul(at_ps[:, 1, 128:256],
                                         lhsT=ks[pb:pb + 64, blk, 128:256],
                                         rhs=qs[pb:pb + 64, blk, 128:256],
                                         start=True, stop=True)
                        a_sb = a_pool.tile([128, 2, C], BF16, tag="a",
                                           name=f"a{ch}_{h}")
                        nc.vector.tensor_mul(a_sb, at_ps, jtm)
                        a_sbs[h] = a_sb

                    def emit_o(h):
                        # intra-chunk matmuls first; the o_inter matmul last
                        # so the cross-chunk state dependency lands as late
                        # as possible in the PE queue
                        p, hh = h // 2, h % 2
                        blk, pb = h // 2, (h % 2) * 64
                        o_slice = o_ps_pairs[p][:, hh, :]
                        vl = v_sb[:, vtb0, h * 128:(h + 1) * 128]
                        a_sb = a_sbs[h]
                        if ch > 0:
                            # inter first, jb1, then one merged full-width
                            # jb0 matmul carrying the single stop (last
                            # writer of both psum halves)
                            nc.tensor.matmul(
                                o_slice,
                                lhsT=s_sb[hh * 64:hh * 64 + 64, p,
                                          hh * 128:(hh + 1) * 128],
                                rhs=qs[pb:pb + 64, blk, :],
                                start=True, stop=False)
                            nc.tensor.matmul(o_slice[:, 128:256],
                                             lhsT=v_sb[:, vtb0 + 1,
                                                       h * 128:(h + 1) * 128],
                                             rhs=a_sb[:, 1, 128:256],
                                             start=False, stop=False)
                            nc.tensor.matmul(o_slice, lhsT=vl,
                                             rhs=a_sb[:, 0, :],
                                             start=False, stop=True)
                        else:
                            nc.tensor.matmul(o_slice[:, 0:128], lhsT=vl,
                                             rhs=a_sb[:, 0, 0:128],
                                             start=True, stop=True)
                            nc.tensor.matmul(o_slice[:, 128:256], lhsT=vl,
                                             rhs=a_sb[:, 0, 128:256],
                                             start=True, stop=False)
                            nc.tensor.matmul(o_slice[:, 128:256],
                                             lhsT=v_sb[:, vtb0 + 1,
                                                       h * 128:(h + 1) * 128],
                                             rhs=a_sb[:, 1, 128:256],
                                             start=False, stop=True)

                    def norm_front(p):
                        """scalar/vector front half of the norm chain."""
                        o_ps = o_ps_pairs[p]
                        o_flat = o_ps.rearrange('p a b -> p (a b)')
                        osc = nrm_pool.tile([128, 512], F32, tag="osc",
                                            name=f"osc{ch}_{p}")
                        nc.scalar.copy(osc, o_flat)
                        o2 = nrm_pool.tile([128, 512], BF16, tag="o2",
                                           name=f"o2{ch}_{p}")
                        nc.vector.tensor_mul(o2, osc, o_flat)
                        return osc, o2

                    def norm_mid(p, osc, o2):
                        """mean matmul + rsqrt + partition broadcast."""
                        mean_ps = ps_small.tile([1, 512], F32, tag="small",
                                                name=f"mean{ch}_{p}")
                        nc.tensor.matmul(mean_ps, lhsT=ones, rhs=o2,
                                         start=True, stop=True)
                        rsq1 = nrm_pool.tile([1, 512], F32, tag="rsq1",
                                             name=f"rsq{ch}_{p}")
                        nc.scalar.activation(rsq1, mean_ps,
                                             AF.Abs_reciprocal_sqrt,
                                             bias=epsb, scale=1.0 / DV)
                        rsqf = nrm_pool.tile([128, 512], F32, tag="rsqf",
                                             name=f"rsqf{ch}_{p}")
                        nc.gpsimd.partition_broadcast(rsqf, rsq1)
                        return rsqf

                    emit_at(0)
                    emit_at(1)

                    # state-update matmuls early: inputs (kdkv, v) are ready
                    # at chunk start, and the s_sb write then lands well
                    # before the next chunk's o_inter reads it
                    ds_ps = ps_small.tile([128, 2, C], F32, tag="small",
                                          name=f"ds{ch}")
                    for p in range(2):
                        for jb in range(2):
                            nc.tensor.matmul(
                                ds_ps[:, p, :],
                                lhsT=kdkv[:, jb, p * 128:(p + 1) * 128],
                                rhs=v_sb[:, vtb0 + jb, p * 256:(p + 1) * 256],
                                start=(jb == 0), stop=(jb == 1))

                    emit_o(0)
                    emit_at(2)
                    emit_o(1)            # pair 0 o_ps complete
                    nf0 = norm_front(0)
                    emit_at(3)
                    emit_o(2)
                    emit_o(HPC - 1)      # pair 1 o_ps complete

                    for p in range(2):
                        nc.vector.scalar_tensor_tensor(
                            out=s_sb[:, p, :],
                            in0=s_sb[:, p, :],
                            scalar=gcv[:, p:p + 1],
                            in1=ds_ps[:, p, :],
                            op0=mybir.AluOpType.mult,
                            op1=mybir.AluOpType.add)
                    nf1 = norm_front(1)
                    if ch == NCH - 1:
                        rsqf1 = norm_mid(1, *nf1)
                    rsqf0 = norm_mid(0, *nf0)

                    # deferred norm tail + output projection of older
                    # chunks (two-chunk deferral: og is never the limiter)
                    while pending_wo:
                        pnf0, pnf1, prs0, prs1, pgsil, pch, pcc, pc0 = \
                            pending_wo.pop(0)
                        og_prev = [norm_tail_p(0, pnf0[0], prs0, pgsil,
                                               pch, pcc),
                                   norm_tail_p(1, pnf1[0], prs1, pgsil,
                                               pch, pcc)]
                        emit_wo(og_prev, pc0)
                    if ch != NCH - 1:
                        rsqf1 = norm_mid(1, *nf1)

                    if ch == NCH - 1:
                        og_pairs = [norm_tail_p(0, nf0[0], rsqf0, gsil,
                                                ch, cc),
                                    norm_tail_p(1, nf1[0], rsqf1, gsil,
                                                ch, cc)]
                        emit_wo(og_pairs, c0, last=True)
                    else:
                        pending_wo.append((nf0, nf1, rsqf0, rsqf1, gsil,
                                           ch, cc, c0))

    nc.finalize()
    return nc


def _host_tables(heads):
    """Per-core constant tables for a 4-head slice."""
    gam = (1.0 - 2.0 ** (-5.0 - np.arange(H, dtype=np.float64)))[heads]  # [4]

    # rope tables with retention decay folded in, feature-major [128, 4, T]
    # (cos m0, cos m1, sin m0, sin m1); rows permuted by P64 within each
    # 64-dim head block so rotate partners share a 32-partition quadrant
    inv = 10000.0 ** (-np.arange(0, DK, 2, dtype=np.float64) / DK)  # [32]
    t_idx = np.arange(T, dtype=np.float64)
    ang = np.outer(t_idx, inv)                      # [T, 32]
    cos_t, sin_t = np.cos(ang), np.sin(ang)         # [T, 32]
    i_in_chunk = (np.arange(T) % C).astype(np.float64)

    CSQ = np.empty((128, 4, T), np.float64)
    CSK = np.empty((128, 4, T), np.float64)
    for m in range(2):
        for p in range(128):
            f = m * 128 + p            # feature index within the 4-head slice
            hc = f // 64               # head-local index 0..3
            dd = P64[f % 64]           # original dim within the head
            idx = dd % 32
            sign = 1.0 if dd < 32 else -1.0
            dq = gam[hc] ** (i_in_chunk + 1.0)
            dk = gam[hc] ** (-i_in_chunk - 1.0)
            CSQ[p, m, :] = cos_t[:, idx] * dq
            CSQ[p, 2 + m, :] = sign * sin_t[:, idx] * dq
            CSK[p, m, :] = cos_t[:, idx] * dk
            CSK[p, 2 + m, :] = sign * sin_t[:, idx] * dk

    # full-tile A mask [128(j), 2(jb), 256(i)]: jb0 = [tri | ones]
    # (diagonal block then unmasked off-diagonal), jb1 = [zero | tri]
    # (the never-read strictly-upper quarter stays zeroed)
    j_idx = np.arange(128)
    tri = (j_idx[:, None] <= j_idx[None, :]).astype(np.float32)
    on = np.ones((128, 128), np.float32)
    JTt = np.stack([np.concatenate([tri, on], 1),
                    np.concatenate([0 * on, tri], 1)], axis=1)

    # gamma^C per k-token-major column (column c -> head c//64)
    GKCt = np.broadcast_to(
        np.repeat(gam ** C, 64)[None, :], (128, 256)).astype(np.float32)

    # gamma^C per state-pair row
    GCVt = np.empty((128, 2), np.float32)
    for p in range(2):
        GCVt[0:64, p] = gam[2 * p] ** C
        GCVt[64:128, p] = gam[2 * p + 1] ** C
    return CSQ, CSK, JTt, GKCt, GCVt


def _prepare_inputs(x, Wq, Wk, Wv, Wg, Wo, g_norm_w):
    x = np.asarray(x, np.float32)
    Wq = np.asarray(Wq, np.float32) * (DK ** -0.5)
    Wk = np.asarray(Wk, np.float32)
    Wv = np.asarray(Wv, np.float32)
    Wg = np.asarray(Wg, np.float32)
    Wo = np.asarray(Wo, np.float32)
    gw = np.asarray(g_norm_w, np.float32)

    in_maps = []
    for core in range(NCORES):
        b = core // 4
        hg = core % 4
        heads = np.arange(4 * hg, 4 * hg + 4)
        # q/k columns with the P64 row permutation applied per head
        qk_cols = np.concatenate(
            [h * DK + np.asarray(P64) for h in heads])
        vg_cols = np.concatenate([np.arange(h * DV, (h + 1) * DV) for h in heads])

        XTc = np.ascontiguousarray(x[b].T).astype(BF)
        WQc = np.ascontiguousarray(
            Wq[:, qk_cols].reshape(8, 128, 256).transpose(1, 0, 2)).astype(BF)
        WKc = np.ascontiguousarray(
            Wk[:, qk_cols].reshape(8, 128, 256).transpose(1, 0, 2)).astype(BF)
        WVc = np.ascontiguousarray(
            Wv[:, vg_cols].reshape(8, 128, 512).transpose(1, 0, 2)).astype(BF)
        WGc = np.ascontiguousarray(
            Wg[:, vg_cols].reshape(8, 128, 512).transpose(1, 0, 2)).astype(BF)
        WOc = np.ascontiguousarray(
            (Wo[vg_cols, :] * np.tile(gw, 4)[:, None])
            .reshape(4, 128, 1024).transpose(1, 0, 2)).astype(BF)

        CSQt, CSKt, JTt, GKCt, GCVt = _host_tables(heads)

        in_maps.append({
            "XT": XTc, "WQ": WQc, "WK": WKc, "WV": WVc, "WG": WGc, "WO": WOc,
            "CSQ": CSQt.astype(BF), "CSK": CSKt.astype(BF),
            "JT": JTt.astype(BF), "GKC": GKCt,
            "GCV": GCVt,
            "ONES": np.ones((128, 1), BF),
            "IDENT": np.eye(128, dtype=BF),
            "ZS": np.zeros((128, 2, C), BF),
        })
    return in_maps


def _run(in_maps, **kw):
    if "nc" not in _cache:
        _cache["nc"] = _build_program()
    return run_bass_kernel_spmd(_cache["nc"], in_maps,
                                core_ids=list(range(NCORES)), **kw)


def kernel(x, Wq, Wk, Wv, Wg, Wo, g_norm_w):
    in_maps = _prepare_inputs(x, Wq, Wk, Wv, Wg, Wo, g_norm_w)
    res = _run(in_maps)
    out = np.zeros((B, T, D), np.float32)
    for core in range(NCORES):
        out[core // 4] += res.results[core]["OUT"].astype(np.float32)
    return out


# revision 22
# speedup vs baseline: 1.2072x; 1.0127x over previous
"""FLARetNet Trainium2 kernel: 8-core SPMD, batch x head-group sharding.

Each core handles one batch (B=2 -> 4 cores per batch) and 4 of 16 heads.
Per core: qkvg projections (fp16 matmuls), neox RoPE, RetNet chunked
retention scan (chunk=256), fused RMSNorm + swish gate, output projection
(partial sum over its heads). Host sums the 4 partials per batch.

Schedule/layout notes:
- Retention decay is folded into the RoPE tables: q tables carry
  gamma^(i+1), k tables carry gamma^(-j-1) (i,j = position in chunk).
  Off-diagonal A blocks then need no mask at all; only the two 128x128
  diagonal blocks get a 0/1 triangular mask, and the strictly-upper
  block is never computed. kdkv becomes a constant gamma^C column scale.
- RoPE rotate-half runs as a DVE stream_shuffle: the q/k feature rows
  are permuted host-side (within each head's 64 dims) so rotation
  partners sit in the same 32-partition quadrant.
- Matmuls run fp16 (full PE rate); PSUM accumulation is fp32.
- The per-chunk Wo projection is deferred by one chunk and split into
  per-og-pair passes so it never waits on the full norm/gate chain.
- The rsqrt activation table is pre-warmed with a dummy op after each
  projection tile's silus so the table load stays off the norm chain.
"""
import numpy as np
import ml_dtypes

import concourse.mybir as mybir
import concourse.tile as tile
import concourse.bacc as bacc
import concourse.bass_isa as bass_isa
from concourse.bass_utils import run_bass_kernel_spmd

F32 = mybir.dt.float32
BF16 = mybir.dt.float16
AF = mybir.ActivationFunctionType
BF = np.float16

B, T, D, H = 2, 4096, 1024, 16
DK, DV = 64, 128
C = 256            # attention chunk length (math-equivalent for any C)
PT = 512           # projection token-tile
NCH = T // C       # 16 chunks
HPC = 4            # heads per core
NCORES = 8

# rope-partner shuffle: within each 32-partition quadrant swap halves
SHUF = list(range(16, 32)) + list(range(16))
# row permutation within each 64-dim head block so partners share a quadrant
P64 = list(range(16)) + list(range(32, 48)) + list(range(16, 32)) + list(range(48, 64))

_cache = {}


def _build_program():
    nc = bacc.Bacc("TRN2", target_bir_lowering=False, debug=False)

    XT = nc.dram_tensor("XT", [D, T], BF16, kind="ExternalInput")
    WQ = nc.dram_tensor("WQ", [128, 8, 256], BF16, kind="ExternalInput")
    WK = nc.dram_tensor("WK", [128, 8, 256], BF16, kind="ExternalInput")
    WV = nc.dram_tensor("WV", [128, 8, 512], BF16, kind="ExternalInput")
    WG = nc.dram_tensor("WG", [128, 8, 512], BF16, kind="ExternalInput")
    WO = nc.dram_tensor("WO", [128, 4, 1024], BF16, kind="ExternalInput")
    CSQ = nc.dram_tensor("CSQ", [128, 4, T], BF16, kind="ExternalInput")
    CSK = nc.dram_tensor("CSK", [128, 4, T], BF16, kind="ExternalInput")
    JT = nc.dram_tensor("JT", [128, 2, 256], BF16, kind="ExternalInput")
    GKC = nc.dram_tensor("GKC", [128, 256], F32, kind="ExternalInput")
    GCV = nc.dram_tensor("GCV", [128, 2], F32, kind="ExternalInput")
    IDENT = nc.dram_tensor("IDENT", [128, 128], BF16, kind="ExternalInput")
    ONES = nc.dram_tensor("ONES", [128, 1], BF16, kind="ExternalInput")
    ZS = nc.dram_tensor("ZS", [128, 2, C], BF16, kind="ExternalInput")

    OUT = nc.dram_tensor("OUT", [T, D], BF16, kind="ExternalOutput")

    with tile.TileContext(nc) as tc:
        with tc.tile_pool(name="singles", bufs=1) as singles, \
             tc.tile_pool(name="xt", bufs=2) as xt_pool, \
             tc.tile_pool(name="tab", bufs=2) as tab_pool, \
             tc.tile_pool(name="rope", bufs=2) as rope_pool, \
             tc.tile_pool(name="qk", bufs=2) as qk_pool, \
             tc.tile_pool(name="vsb", bufs=2) as v_pool, \
             tc.tile_pool(name="asb", bufs=3) as a_pool, \
             tc.tile_pool(name="gat", bufs=2) as g_pool, \
             tc.tile_pool(name="nrm", bufs=4) as nrm_pool, \
             tc.tile_pool(name="og", bufs=4) as og_pool, \
             tc.tile_pool(name="osb", bufs=3) as out_pool, \
             tc.tile_pool(name="ps_proj", bufs=2, space="PSUM") as ps_proj, \
             tc.tile_pool(name="ps_small", bufs=3, space="PSUM") as ps_small, \
             tc.tile_pool(name="ps_o", bufs=3, space="PSUM") as ps_o:

            # ---- resident weights/tables (usage order; wo last) ----
            wq = singles.tile([128, 8, 256], BF16)
            wk = singles.tile([128, 8, 256], BF16)
            wv = singles.tile([128, 8, 512], BF16)
            wg = singles.tile([128, 8, 512], BF16)
            wo = singles.tile([128, 4, 1024], BF16)
            identmm = singles.tile([128, 128], BF16)
            jtm = singles.tile([128, 2, 256], BF16)
            gkc = singles.tile([128, 256], F32)
            gcv = singles.tile([128, 2], F32)
            ones = singles.tile([128, 1], BF16)
            s_sb = singles.tile([128, 2, C], BF16)

            nc.gpsimd.dma_start(out=wq[:, :, 0:128], in_=WQ[:, :, 0:128])
            nc.gpsimd.dma_start(out=wq[:, :, 128:256], in_=WQ[:, :, 128:256])
            nc.gpsimd.dma_start(out=wk, in_=WK[:, :, :])
            nc.gpsimd.dma_start(out=wv, in_=WV[:, :, :])
            nc.gpsimd.dma_start(out=wg, in_=WG[:, :, :])
            nc.gpsimd.dma_start(out=identmm, in_=IDENT[:, :])
            nc.gpsimd.dma_start(out=jtm, in_=JT[:, :, :])
            nc.gpsimd.dma_start(out=gkc, in_=GKC[:, :])
            nc.gpsimd.dma_start(out=gcv, in_=GCV[:, :])
            nc.gpsimd.dma_start(out=ones, in_=ONES[:, :])
            nc.gpsimd.dma_start(out=s_sb, in_=ZS[:, :, :])
            nc.gpsimd.dma_start(out=wo, in_=WO[:, :, :])

            epsb = singles.tile([1, 1], F32)
            nc.vector.memset(epsb, 1e-5)

            def emit_wo(og_pair_list, oc0, last=False):
                for tb in range(2):
                    out_ps = [ps_small.tile([128, 512], F32, tag="small",
                                            name=f"wo{oc0}_{tb}_{nn}")
                              for nn in range(2)]
                    # pair-0 heads first so this can start before pair-1's
                    # norm chain finishes; in the last chunk, finish nn=0
                    # completely first so its copy/DMA overlaps nn=1's mms
                    def pass0(nn):
                        for hh in range(2):
                            nc.tensor.matmul(
                                out_ps[nn],
                                lhsT=og_pair_list[0][
                                    :, hh, tb * 128:(tb + 1) * 128],
                                rhs=wo[:, hh, nn * 512:(nn + 1) * 512],
                                start=(hh == 0), stop=False)
                    def pass1(nn):
                        for hh in range(2):
                            nc.tensor.matmul(
                                out_ps[nn],
                                lhsT=og_pair_list[1][
                                    :, hh, tb * 128:(tb + 1) * 128],
                                rhs=wo[:, 2 + hh, nn * 512:(nn + 1) * 512],
                                start=False, stop=(hh == 1))
                    def flush(nn, eng):
                        out_sb = out_pool.tile([128, 512], BF16, tag="outsb",
                                               name=f"wos{oc0}_{tb}_{nn}")
                        if eng == "v":
                            nc.vector.tensor_copy(out=out_sb, in_=out_ps[nn])
                        else:
                            nc.scalar.copy(out_sb, out_ps[nn])
                        nc.sync.dma_start(
                            out=OUT[oc0 + tb * 128:oc0 + (tb + 1) * 128,
                                    nn * 512:(nn + 1) * 512],
                            in_=out_sb)
                    if last:
                        pass0(0)
                        pass1(0)
                        flush(0, "s")
                        pass0(1)
                        pass1(1)
                        flush(1, "v")
                    else:
                        pass0(0)
                        pass0(1)
                        pass1(0)
                        pass1(1)
                        flush(0, "s")
                        flush(1, "s")

            pending_wo = []

            def norm_tail_p(p, osc, rsqf, gsil_t, cch, ccc):
                onrm = nrm_pool.tile([128, 512], F32, tag="onrm",
                                     name=f"onrm{cch}_{p}")
                nc.vector.tensor_mul(onrm, osc, rsqf)
                og = og_pool.tile([128, 2, C], BF16, tag="og",
                                  name=f"og{cch}_{p}")
                gs = gsil_t[:, p * 2:(p + 1) * 2, ccc * C:(ccc + 1) * C]
                nc.vector.tensor_mul(
                    og, onrm.rearrange('p (a b) -> p a b', a=2), gs)
                return og

            for pt in range(T // PT):
                p0 = pt * PT

                xt = xt_pool.tile([128, 8, PT], BF16, tag="xt")
                xt_src = XT.rearrange("(db p) t -> p db t", p=128)
                for xq in range(4):
                    eng = nc.sync if xq % 2 == 0 else nc.scalar
                    eng.dma_start(out=xt[:, 2 * xq:2 * xq + 2, :],
                                  in_=xt_src[:, 2 * xq:2 * xq + 2,
                                             p0:p0 + PT])

                csq = tab_pool.tile([128, 4, PT], BF16, tag="csq")
                nc.sync.dma_start(out=csq, in_=CSQ[:, :, p0:p0 + PT])
                csk = tab_pool.tile([128, 4, PT], BF16, tag="csk")
                nc.scalar.dma_start(out=csk, in_=CSK[:, :, p0:p0 + PT])

                # ---- projections over PT tokens ----
                # q, k feature-major [128(dim%128), blk, tok]; RoPE fused and
                # the retention decay folded into the cos/sin tables
                def proj_rope(w, cs, tag):
                    out = qk_pool.tile([128, 2, PT], BF16, tag=tag,
                                       name=f"{tag}{pt}")
                    for m in range(2):
                        pps = ps_proj.tile([128, PT], F32, tag="proj",
                                           name=f"{tag}ps{pt}_{m}")
                        for db in range(8):
                            nc.tensor.matmul(
                                pps, lhsT=w[:, db, m * 128:(m + 1) * 128],
                                rhs=xt[:, db, :],
                                start=(db == 0), stop=(db == 7))
                        tcos = rope_pool.tile([128, PT], BF16, tag="tcos")
                        tsin = rope_pool.tile([128, PT], BF16, tag="tsin")
                        rot = rope_pool.tile([128, PT], BF16, tag="rot")
                        nc.vector.tensor_mul(tcos, pps, cs[:, m, :])
                        nc.vector.tensor_mul(tsin, pps, cs[:, 2 + m, :])
                        nc.vector.stream_shuffle(rot, tsin, SHUF)
                        nc.vector.tensor_add(out[:, m, :], tcos, rot)
                    return out

                q_sb = proj_rope(wq, csq, "q")   # gamma^(i+1) q, fp16
                k_sb = proj_rope(wk, csk, "k")   # gamma^(-j-1) k, fp16

                # v token-major [128(tok%128), tb, dim]
                v_sb = v_pool.tile([128, 4, 512], BF16, tag="v")
                for tb in range(4):
                    v_ps = ps_proj.tile([128, 512], F32, tag="proj",
                                        name=f"vps{pt}_{tb}")
                    for db in range(8):
                        nc.tensor.matmul(
                            v_ps,
                            lhsT=xt[:, db, tb * 128:(tb + 1) * 128],
                            rhs=wv[:, db, :],
                            start=(db == 0), stop=(db == 7))
                    nc.scalar.copy(v_sb[:, tb, :], v_ps)

                # g feature-major per head-block -> silu
                gsil = g_pool.tile([128, 4, PT], BF16, tag="gsil")
                for m in range(4):
                    g_ps = ps_proj.tile([128, PT], F32, tag="proj",
                                        name=f"gps{pt}_{m}")
                    for db in range(8):
                        nc.tensor.matmul(
                            g_ps, lhsT=wg[:, db, m * 128:(m + 1) * 128],
                            rhs=xt[:, db, :],
                            start=(db == 0), stop=(db == 7))
                    nc.scalar.activation(gsil[:, m, :], g_ps, AF.Silu)

                # ---- per 256-chunk attention ----
                for cc in range(PT // C):
                    ch = pt * (PT // C) + cc
                    c0 = ch * C
                    qs = q_sb[:, :, cc * C:(cc + 1) * C]
                    ks = k_sb[:, :, cc * C:(cc + 1) * C]
                    vtb0 = cc * 2

                    # k token-major + gamma^C scaling (for the state update)
                    ktm_ps = ps_small.tile([128, 2, C], BF16, tag="small",
                                           name=f"ktm{ch}")
                    for tb in range(2):
                        for b in range(2):
                            nc.tensor.transpose(
                                ktm_ps[:, tb, b * 128:(b + 1) * 128],
                                ks[:, b, tb * 128:(tb + 1) * 128],
                                identmm)
                    kdkv = qk_pool.tile([128, 2, C], BF16, tag="kdkv")
                    nc.vector.tensor_mul(
                        kdkv, ktm_ps,
                        gkc[:, None, :].broadcast_to([128, 2, C]))

                    o_ps_pairs = [ps_o.tile([128, 2, C], F32, tag="o",
                                            name=f"o_ps{ch}_{i}")
                                  for i in range(2)]
                    a_sbs = [None] * HPC

                    def emit_at(h):
                        blk, pb = h // 2, (h % 2) * 64
                        at_ps = ps_small.tile([128, 2, C], F32, tag="small",
                                              name=f"at{ch}_{h}")
                        nc.tensor.matmul(at_ps[:, 0, :],
                                         lhsT=ks[pb:pb + 64, blk, 0:128],
                                         rhs=qs[pb:pb + 64, blk, :],
                                         start=True, stop=True)
                        nc.tensor.matmul(at_ps[:, 1, 128:256],
                                         lhsT=ks[pb:pb + 64, blk, 128:256],
                                         rhs=qs[pb:pb + 64, blk, 128:256],
                                         start=True, stop=True)
                        a_sb = a_pool.tile([128, 2, C], BF16, tag="a",
                                           name=f"a{ch}_{h}")
                        nc.vector.tensor_mul(a_sb, at_ps, jtm)
                        a_sbs[h] = a_sb

                    def emit_o(h):
                        # intra-chunk matmuls first; the o_inter matmul last
                        # so the cross-chunk state dependency lands as late
                        # as possible in the PE queue
                        p, hh = h // 2, h % 2
                        blk, pb = h // 2, (h % 2) * 64
                        o_slice = o_ps_pairs[p][:, hh, :]
                        vl = v_sb[:, vtb0, h * 128:(h + 1) * 128]
                        a_sb = a_sbs[h]
                        if ch > 0:
                            # inter first, jb1, then one merged full-width
                            # jb0 matmul carrying the single stop (last
                            # writer of both psum halves)
                            nc.tensor.matmul(
                                o_slice,
                                lhsT=s_sb[hh * 64:hh * 64 + 64, p,
                                          hh * 128:(hh + 1) * 128],
                                rhs=qs[pb:pb + 64, blk, :],
                                start=True, stop=False)
                            nc.tensor.matmul(o_slice[:, 128:256],
                                             lhsT=v_sb[:, vtb0 + 1,
                                                       h * 128:(h + 1) * 128],
                                             rhs=a_sb[:, 1, 128:256],
                                             start=False, stop=False)
                            nc.tensor.matmul(o_slice, lhsT=vl,
                                             rhs=a_sb[:, 0, :],
                                             start=False, stop=True)
                        else:
                            nc.tensor.matmul(o_slice[:, 0:128], lhsT=vl,
                                             rhs=a_sb[:, 0, 0:128],
                                             start=True, stop=True)
                            nc.tensor.matmul(o_slice[:, 128:256], lhsT=vl,
                                             rhs=a_sb[:, 0, 128:256],
                                             start=True, stop=False)
                            nc.tensor.matmul(o_slice[:, 128:256],
                                             lhsT=v_sb[:, vtb0 + 1,
                                                       h * 128:(h + 1) * 128],
                                             rhs=a_sb[:, 1, 128:256],
                                             start=False, stop=True)

                    def norm_front(p):
                        """scalar/vector front half of the norm chain."""
                        o_ps = o_ps_pairs[p]
                        o_flat = o_ps.rearrange('p a b -> p (a b)')
                        osc = nrm_pool.tile([128, 512], F32, tag="osc",
                                            name=f"osc{ch}_{p}")
                        nc.scalar.copy(osc, o_flat)
                        o2 = nrm_pool.tile([128, 512], BF16, tag="o2",
                                           name=f"o2{ch}_{p}")
                        nc.vector.tensor_mul(o2, osc, o_flat)
                        return osc, o2

                    def norm_mid(p, osc, o2):
                        """mean matmul + rsqrt + partition broadcast."""
                        mean_ps = ps_small.tile([1, 512], F32, tag="small",
                                                name=f"mean{ch}_{p}")
                        nc.tensor.matmul(mean_ps, lhsT=ones, rhs=o2,
                                         start=True, stop=True)
                        rsq1 = nrm_pool.tile([1, 512], F32, tag="rsq1",
                                             name=f"rsq{ch}_{p}")
                        nc.scalar.activation(rsq1, mean_ps,
                                             AF.Abs_reciprocal_sqrt,
                                             bias=epsb, scale=1.0 / DV)
                        rsqf = nrm_pool.tile([128, 512], F32, tag="rsqf",
                                             name=f"rsqf{ch}_{p}")
                        nc.gpsimd.partition_broadcast(rsqf, rsq1)
                        return rsqf

                    emit_at(0)
                    emit_at(1)

                    # state-update matmuls early: inputs (kdkv, v) are ready
                    # at chunk start, and the s_sb write then lands well
                    # before the next chunk's o_inter reads it
                    ds_ps = ps_small.tile([128, 2, C], F32, tag="small",
                                          name=f"ds{ch}")
                    for p in range(2):
                        for jb in range(2):
                            nc.tensor.matmul(
                                ds_ps[:, p, :],
                                lhsT=kdkv[:, jb, p * 128:(p + 1) * 128],
                                rhs=v_sb[:, vtb0 + jb, p * 256:(p + 1) * 256],
                                start=(jb == 0), stop=(jb == 1))

                    emit_o(0)
                    emit_at(2)
                    emit_o(1)            # pair 0 o_ps complete
                    nf0 = norm_front(0)
                    emit_at(3)
                    emit_o(2)
                    emit_o(HPC - 1)      # pair 1 o_ps complete

                    for p in range(2):
                        nc.vector.scalar_tensor_tensor(
                            out=s_sb[:, p, :],
                            in0=s_sb[:, p, :],
                            scalar=gcv[:, p:p + 1],
                            in1=ds_ps[:, p, :],
                            op0=mybir.AluOpType.mult,
                            op1=mybir.AluOpType.add)
                    nf1 = norm_front(1)
                    if ch == NCH - 1:
                        rsqf1 = norm_mid(1, *nf1)
                    rsqf0 = norm_mid(0, *nf0)

                    # deferred norm tail + output projection of older
                    # chunks (two-chunk deferral: og is never the limiter)
                    while pending_wo and not (ch == 1 and
                                               pending_wo[0][5] == 0):
                        pnf0, pnf1, prs0, prs1, pgsil, pch, pcc, pc0 = \
                            pending_wo.pop(0)
                        og_prev = [norm_tail_p(0, pnf0[0], prs0, pgsil,
                                               pch, pcc),
                                   norm_tail_p(1, pnf1[0], prs1, pgsil,
                                               pch, pcc)]
                        emit_wo(og_prev, pc0)
                    if ch != NCH - 1:
                        rsqf1 = norm_mid(1, *nf1)

                    if ch == NCH - 1:
                        og_pairs = [norm_tail_p(0, nf0[0], rsqf0, gsil,
                                                ch, cc),
                                    norm_tail_p(1, nf1[0], rsqf1, gsil,
                                                ch, cc)]
                        emit_wo(og_pairs, c0, last=True)
                    else:
                        pending_wo.append((nf0, nf1, rsqf0, rsqf1, gsil,
                                           ch, cc, c0))

    nc.finalize()
    return nc


def _host_tables(heads):
    """Per-core constant tables for a 4-head slice."""
    gam = (1.0 - 2.0 ** (-5.0 - np.arange(H, dtype=np.float64)))[heads]  # [4]

    # rope tables with retention decay folded in, feature-major [128, 4, T]
    # (cos m0, cos m1, sin m0, sin m1); rows permuted by P64 within each
    # 64-dim head block so rotate partners share a 32-partition quadrant
    inv = 10000.0 ** (-np.arange(0, DK, 2, dtype=np.float64) / DK)  # [32]
    t_idx = np.arange(T, dtype=np.float64)
    ang = np.outer(t_idx, inv)                      # [T, 32]
    cos_t, sin_t = np.cos(ang), np.sin(ang)         # [T, 32]
    i_in_chunk = (np.arange(T) % C).astype(np.float64)

    CSQ = np.empty((128, 4, T), np.float64)
    CSK = np.empty((128, 4, T), np.float64)
    for m in range(2):
        for p in range(128):
            f = m * 128 + p            # feature index within the 4-head slice
            hc = f // 64               # head-local index 0..3
            dd = P64[f % 64]           # original dim within the head
            idx = dd % 32
            sign = 1.0 if dd < 32 else -1.0
            dq = gam[hc] ** (i_in_chunk + 1.0)
            dk = gam[hc] ** (-i_in_chunk - 1.0)
            CSQ[p, m, :] = cos_t[:, idx] * dq
            CSQ[p, 2 + m, :] = sign * sin_t[:, idx] * dq
            CSK[p, m, :] = cos_t[:, idx] * dk
            CSK[p, 2 + m, :] = sign * sin_t[:, idx] * dk

    # full-tile A mask [128(j), 2(jb), 256(i)]: jb0 = [tri | ones]
    # (diagonal block then unmasked off-diagonal), jb1 = [zero | tri]
    # (the never-read strictly-upper quarter stays zeroed)
    j_idx = np.arange(128)
    tri = (j_idx[:, None] <= j_idx[None, :]).astype(np.float32)
    on = np.ones((128, 128), np.float32)
    JTt = np.stack([np.concatenate([tri, on], 1),
                    np.concatenate([0 * on, tri], 1)], axis=1)

    # gamma^C per k-token-major column (column c -> head c//64)
    GKCt = np.broadcast_to(
        np.repeat(gam ** C, 64)[None, :], (128, 256)).astype(np.float32)

    # gamma^C per state-pair row
    GCVt = np.empty((128, 2), np.float32)
    for p in range(2):
        GCVt[0:64, p] = gam[2 * p] ** C
        GCVt[64:128, p] = gam[2 * p + 1] ** C
    return CSQ, CSK, JTt, GKCt, GCVt


def _prepare_inputs(x, Wq, Wk, Wv, Wg, Wo, g_norm_w):
    x = np.asarray(x, np.float32)
    Wq = np.asarray(Wq, np.float32) * (DK ** -0.5)
    Wk = np.asarray(Wk, np.float32)
    Wv = np.asarray(Wv, np.float32)
    Wg = np.asarray(Wg, np.float32)
    Wo = np.asarray(Wo, np.float32)
    gw = np.asarray(g_norm_w, np.float32)

    in_maps = []
    for core in range(NCORES):
        b = core // 4
        hg = core % 4
        heads = np.arange(4 * hg, 4 * hg + 4)
        # q/k columns with the P64 row permutation applied per head
        qk_cols = np.concatenate(
            [h * DK + np.asarray(P64) for h in heads])
        vg_cols = np.concatenate([np.arange(h * DV, (h + 1) * DV) for h in heads])

        XTc = np.ascontiguousarray(x[b].T).astype(BF)
        WQc = np.ascontiguousarray(
            Wq[:, qk_cols].reshape(8, 128, 256).transpose(1, 0, 2)).astype(BF)
        WKc = np.ascontiguousarray(
            Wk[:, qk_cols].reshape(8, 128, 256).transpose(1, 0, 2)).astype(BF)
        WVc = np.ascontiguousarray(
            Wv[:, vg_cols].reshape(8, 128, 512).transpose(1, 0, 2)).astype(BF)
        WGc = np.ascontiguousarray(
            Wg[:, vg_cols].reshape(8, 128, 512).transpose(1, 0, 2)).astype(BF)
        WOc = np.ascontiguousarray(
            (Wo[vg_cols, :] * np.tile(gw, 4)[:, None])
            .reshape(4, 128, 1024).transpose(1, 0, 2)).astype(BF)

        CSQt, CSKt, JTt, GKCt, GCVt = _host_tables(heads)

        in_maps.append({
            "XT": XTc, "WQ": WQc, "WK": WKc, "WV": WVc, "WG": WGc, "WO": WOc,
            "CSQ": CSQt.astype(BF), "CSK": CSKt.astype(BF),
            "JT": JTt.astype(BF), "GKC": GKCt,
            "GCV": GCVt,
            "ONES": np.ones((128, 1), BF),
            "IDENT": np.eye(128, dtype=BF),
            "ZS": np.zeros((128, 2, C), BF),
        })
    return in_maps


def _run(in_maps, **kw):
    if "nc" not in _cache:
        _cache["nc"] = _build_program()
    return run_bass_kernel_spmd(_cache["nc"], in_maps,
                                core_ids=list(range(NCORES)), **kw)


def kernel(x, Wq, Wk, Wv, Wg, Wo, g_norm_w):
    in_maps = _prepare_inputs(x, Wq, Wk, Wv, Wg, Wo, g_norm_w)
    res = _run(in_maps)
    out = np.zeros((B, T, D), np.float32)
    for core in range(NCORES):
        out[core // 4] += res.results[core]["OUT"].astype(np.float32)
    return out


# revision 23
# speedup vs baseline: 1.2222x; 1.0125x over previous
"""FLARetNet Trainium2 kernel: 8-core SPMD, batch x head-group sharding.

Each core handles one batch (B=2 -> 4 cores per batch) and 4 of 16 heads.
Per core: qkvg projections (fp16 matmuls), neox RoPE, RetNet chunked
retention scan (chunk=256), fused RMSNorm + swish gate, output projection
(partial sum over its heads). Host sums the 4 partials per batch.

Schedule/layout notes:
- Retention decay is folded into the RoPE tables: q tables carry
  gamma^(i+1), k tables carry gamma^(-j-1) (i,j = position in chunk).
  Off-diagonal A blocks then need no mask at all; only the two 128x128
  diagonal blocks get a 0/1 triangular mask, and the strictly-upper
  block is never computed. kdkv becomes a constant gamma^C column scale.
- RoPE rotate-half runs as a DVE stream_shuffle: the q/k feature rows
  are permuted host-side (within each head's 64 dims) so rotation
  partners sit in the same 32-partition quadrant.
- Matmuls run fp16 (full PE rate); PSUM accumulation is fp32.
- The per-chunk Wo projection is deferred by one chunk and split into
  per-og-pair passes so it never waits on the full norm/gate chain.
- The rsqrt activation table is pre-warmed with a dummy op after each
  projection tile's silus so the table load stays off the norm chain.
"""
import numpy as np
import ml_dtypes

import concourse.mybir as mybir
import concourse.tile as tile
import concourse.bacc as bacc
import concourse.bass_isa as bass_isa
from concourse.bass_utils import run_bass_kernel_spmd

F32 = mybir.dt.float32
BF16 = mybir.dt.float16
AF = mybir.ActivationFunctionType
BF = np.float16

B, T, D, H = 2, 4096, 1024, 16
DK, DV = 64, 128
C = 256            # attention chunk length (math-equivalent for any C)
PT = 512           # projection token-tile
NCH = T // C       # 16 chunks
HPC = 4            # heads per core
NCORES = 8

# rope-partner shuffle: within each 32-partition quadrant swap halves
SHUF = list(range(16, 32)) + list(range(16))
# row permutation within each 64-dim head block so partners share a quadrant
P64 = list(range(16)) + list(range(32, 48)) + list(range(16, 32)) + list(range(48, 64))

_cache = {}


def _build_program():
    nc = bacc.Bacc("TRN2", target_bir_lowering=False, debug=False)

    XT = nc.dram_tensor("XT", [D, T], BF16, kind="ExternalInput")
    WQ = nc.dram_tensor("WQ", [128, 8, 256], BF16, kind="ExternalInput")
    WK = nc.dram_tensor("WK", [128, 8, 256], BF16, kind="ExternalInput")
    WV = nc.dram_tensor("WV", [128, 8, 512], BF16, kind="ExternalInput")
    WG = nc.dram_tensor("WG", [128, 8, 512], BF16, kind="ExternalInput")
    WO = nc.dram_tensor("WO", [128, 4, 1024], BF16, kind="ExternalInput")
    CSQ = nc.dram_tensor("CSQ", [128, 4, T], BF16, kind="ExternalInput")
    CSK = nc.dram_tensor("CSK", [128, 4, T], BF16, kind="ExternalInput")
    JT = nc.dram_tensor("JT", [128, 2, 256], BF16, kind="ExternalInput")
    GKC = nc.dram_tensor("GKC", [128, 256], F32, kind="ExternalInput")
    GCV = nc.dram_tensor("GCV", [128, 2], F32, kind="ExternalInput")
    IDENT = nc.dram_tensor("IDENT", [128, 128], BF16, kind="ExternalInput")
    ONES = nc.dram_tensor("ONES", [128, 1], BF16, kind="ExternalInput")
    ZS = nc.dram_tensor("ZS", [128, 2, C], BF16, kind="ExternalInput")

    OUT = nc.dram_tensor("OUT", [T, D], BF16, kind="ExternalOutput")

    with tile.TileContext(nc) as tc:
        with tc.tile_pool(name="singles", bufs=1) as singles, \
             tc.tile_pool(name="xt", bufs=2) as xt_pool, \
             tc.tile_pool(name="tab", bufs=2) as tab_pool, \
             tc.tile_pool(name="rope", bufs=2) as rope_pool, \
             tc.tile_pool(name="qk", bufs=2) as qk_pool, \
             tc.tile_pool(name="vsb", bufs=2) as v_pool, \
             tc.tile_pool(name="asb", bufs=3) as a_pool, \
             tc.tile_pool(name="gat", bufs=2) as g_pool, \
             tc.tile_pool(name="nrm", bufs=4) as nrm_pool, \
             tc.tile_pool(name="og", bufs=4) as og_pool, \
             tc.tile_pool(name="osb", bufs=3) as out_pool, \
             tc.tile_pool(name="ps_proj", bufs=2, space="PSUM") as ps_proj, \
             tc.tile_pool(name="ps_small", bufs=3, space="PSUM") as ps_small, \
             tc.tile_pool(name="ps_o", bufs=3, space="PSUM") as ps_o:

            # ---- resident weights/tables (usage order; wo last) ----
            wq = singles.tile([128, 8, 256], BF16)
            wk = singles.tile([128, 8, 256], BF16)
            wv = singles.tile([128, 8, 512], BF16)
            wg = singles.tile([128, 8, 512], BF16)
            wo = singles.tile([128, 4, 1024], BF16)
            identmm = singles.tile([128, 128], BF16)
            jtm = singles.tile([128, 2, 256], BF16)
            gkc = singles.tile([128, 256], F32)
            gcv = singles.tile([128, 2], F32)
            ones = singles.tile([128, 1], BF16)
            s_sb = singles.tile([128, 2, C], BF16)

            nc.gpsimd.dma_start(out=wq[:, :, 0:128], in_=WQ[:, :, 0:128])
            nc.gpsimd.dma_start(out=wq[:, :, 128:256], in_=WQ[:, :, 128:256])
            nc.gpsimd.dma_start(out=wk, in_=WK[:, :, :])
            nc.gpsimd.dma_start(out=wv, in_=WV[:, :, :])
            nc.gpsimd.dma_start(out=wg, in_=WG[:, :, :])
            nc.gpsimd.dma_start(out=identmm, in_=IDENT[:, :])
            nc.gpsimd.dma_start(out=jtm, in_=JT[:, :, :])
            nc.gpsimd.dma_start(out=gkc, in_=GKC[:, :])
            nc.gpsimd.dma_start(out=s_sb, in_=ZS[:, :, :])
            nc.gpsimd.dma_start(out=wo, in_=WO[:, :, :])
            nc.gpsimd.dma_start(out=gcv, in_=GCV[:, :])
            nc.gpsimd.dma_start(out=ones, in_=ONES[:, :])

            epsb = singles.tile([1, 1], F32)
            nc.vector.memset(epsb, 1e-5)

            def emit_wo(og_pair_list, oc0, last=False):
                for tb in range(2):
                    out_ps = [ps_small.tile([128, 512], F32, tag="small",
                                            name=f"wo{oc0}_{tb}_{nn}")
                              for nn in range(2)]
                    # pair-0 heads first so this can start before pair-1's
                    # norm chain finishes; in the last chunk, finish nn=0
                    # completely first so its copy/DMA overlaps nn=1's mms
                    def pass0(nn):
                        for hh in range(2):
                            nc.tensor.matmul(
                                out_ps[nn],
                                lhsT=og_pair_list[0][
                                    :, hh, tb * 128:(tb + 1) * 128],
                                rhs=wo[:, hh, nn * 512:(nn + 1) * 512],
                                start=(hh == 0), stop=False)
                    def pass1(nn):
                        for hh in range(2):
                            nc.tensor.matmul(
                                out_ps[nn],
                                lhsT=og_pair_list[1][
                                    :, hh, tb * 128:(tb + 1) * 128],
                                rhs=wo[:, 2 + hh, nn * 512:(nn + 1) * 512],
                                start=False, stop=(hh == 1))
                    def flush(nn, eng):
                        out_sb = out_pool.tile([128, 512], BF16, tag="outsb",
                                               name=f"wos{oc0}_{tb}_{nn}")
                        if eng == "v":
                            nc.vector.tensor_copy(out=out_sb, in_=out_ps[nn])
                        else:
                            nc.scalar.copy(out_sb, out_ps[nn])
                        nc.sync.dma_start(
                            out=OUT[oc0 + tb * 128:oc0 + (tb + 1) * 128,
                                    nn * 512:(nn + 1) * 512],
                            in_=out_sb)
                    if last:
                        pass0(0)
                        pass1(0)
                        flush(0, "s")
                        pass0(1)
                        pass1(1)
                        flush(1, "v")
                    else:
                        pass0(0)
                        pass0(1)
                        pass1(0)
                        pass1(1)
                        flush(0, "s")
                        flush(1, "s")

            pending_wo = []

            def norm_tail_p(p, osc, rsqf, gsil_t, cch, ccc):
                onrm = nrm_pool.tile([128, 512], F32, tag="onrm",
                                     name=f"onrm{cch}_{p}")
                nc.vector.tensor_mul(onrm, osc, rsqf)
                og = og_pool.tile([128, 2, C], BF16, tag="og",
                                  name=f"og{cch}_{p}")
                gs = gsil_t[:, p * 2:(p + 1) * 2, ccc * C:(ccc + 1) * C]
                nc.vector.tensor_mul(
                    og, onrm.rearrange('p (a b) -> p a b', a=2), gs)
                return og

            for pt in range(T // PT):
                p0 = pt * PT

                xt = xt_pool.tile([128, 8, PT], BF16, tag="xt")
                xt_src = XT.rearrange("(db p) t -> p db t", p=128)
                for xq in range(4):
                    eng = nc.sync if xq % 2 == 0 else nc.scalar
                    eng.dma_start(out=xt[:, 2 * xq:2 * xq + 2, :],
                                  in_=xt_src[:, 2 * xq:2 * xq + 2,
                                             p0:p0 + PT])

                csq = tab_pool.tile([128, 4, PT], BF16, tag="csq")
                nc.sync.dma_start(out=csq, in_=CSQ[:, :, p0:p0 + PT])
                csk = tab_pool.tile([128, 4, PT], BF16, tag="csk")
                nc.scalar.dma_start(out=csk, in_=CSK[:, :, p0:p0 + PT])

                # ---- projections over PT tokens ----
                # q, k feature-major [128(dim%128), blk, tok]; RoPE fused and
                # the retention decay folded into the cos/sin tables
                def proj_rope(w, cs, tag):
                    out = qk_pool.tile([128, 2, PT], BF16, tag=tag,
                                       name=f"{tag}{pt}")
                    for m in range(2):
                        pps = ps_proj.tile([128, PT], F32, tag="proj",
                                           name=f"{tag}ps{pt}_{m}")
                        for db in range(8):
                            nc.tensor.matmul(
                                pps, lhsT=w[:, db, m * 128:(m + 1) * 128],
                                rhs=xt[:, db, :],
                                start=(db == 0), stop=(db == 7))
                        tcos = rope_pool.tile([128, PT], BF16, tag="tcos")
                        tsin = rope_pool.tile([128, PT], BF16, tag="tsin")
                        rot = rope_pool.tile([128, PT], BF16, tag="rot")
                        nc.vector.tensor_mul(tcos, pps, cs[:, m, :])
                        nc.vector.tensor_mul(tsin, pps, cs[:, 2 + m, :])
                        nc.vector.stream_shuffle(rot, tsin, SHUF)
                        nc.vector.tensor_add(out[:, m, :], tcos, rot)
                    return out

                k_sb = proj_rope(wk, csk, "k")   # gamma^(-j-1) k, fp16
                q_sb = proj_rope(wq, csq, "q")   # gamma^(i+1) q, fp16

                # v token-major [128(tok%128), tb, dim]
                v_sb = v_pool.tile([128, 4, 512], BF16, tag="v")
                for tb in range(4):
                    v_ps = ps_proj.tile([128, 512], F32, tag="proj",
                                        name=f"vps{pt}_{tb}")
                    for db in range(8):
                        nc.tensor.matmul(
                            v_ps,
                            lhsT=xt[:, db, tb * 128:(tb + 1) * 128],
                            rhs=wv[:, db, :],
                            start=(db == 0), stop=(db == 7))
                    nc.scalar.copy(v_sb[:, tb, :], v_ps)

                # g feature-major per head-block -> silu
                gsil = g_pool.tile([128, 4, PT], BF16, tag="gsil")
                for m in range(4):
                    g_ps = ps_proj.tile([128, PT], F32, tag="proj",
                                        name=f"gps{pt}_{m}")
                    for db in range(8):
                        nc.tensor.matmul(
                            g_ps, lhsT=wg[:, db, m * 128:(m + 1) * 128],
                            rhs=xt[:, db, :],
                            start=(db == 0), stop=(db == 7))
                    nc.scalar.activation(gsil[:, m, :], g_ps, AF.Silu)

                # ---- per 256-chunk attention ----
                for cc in range(PT // C):
                    ch = pt * (PT // C) + cc
                    c0 = ch * C
                    qs = q_sb[:, :, cc * C:(cc + 1) * C]
                    ks = k_sb[:, :, cc * C:(cc + 1) * C]
                    vtb0 = cc * 2

                    # k token-major + gamma^C scaling (for the state update)
                    ktm_ps = ps_small.tile([128, 2, C], BF16, tag="small",
                                           name=f"ktm{ch}")
                    for tb in range(2):
                        for b in range(2):
                            nc.tensor.transpose(
                                ktm_ps[:, tb, b * 128:(b + 1) * 128],
                                ks[:, b, tb * 128:(tb + 1) * 128],
                                identmm)
                    kdkv = qk_pool.tile([128, 2, C], BF16, tag="kdkv")
                    nc.vector.tensor_mul(
                        kdkv, ktm_ps,
                        gkc[:, None, :].broadcast_to([128, 2, C]))

                    o_ps_pairs = [ps_o.tile([128, 2, C], F32, tag="o",
                                            name=f"o_ps{ch}_{i}")
                                  for i in range(2)]
                    a_sbs = [None] * HPC

                    def emit_at(h):
                        blk, pb = h // 2, (h % 2) * 64
                        at_ps = ps_small.tile([128, 2, C], F32, tag="small",
                                              name=f"at{ch}_{h}")
                        nc.tensor.matmul(at_ps[:, 0, :],
                                         lhsT=ks[pb:pb + 64, blk, 0:128],
                                         rhs=qs[pb:pb + 64, blk, :],
                                         start=True, stop=True)
                        nc.tensor.matmul(at_ps[:, 1, 128:256],
                                         lhsT=ks[pb:pb + 64, blk, 128:256],
                                         rhs=qs[pb:pb + 64, blk, 128:256],
                                         start=True, stop=True)
                        a_sb = a_pool.tile([128, 2, C], BF16, tag="a",
                                           name=f"a{ch}_{h}")
                        nc.vector.tensor_mul(a_sb, at_ps, jtm)
                        a_sbs[h] = a_sb

                    def emit_o(h):
                        # intra-chunk matmuls first; the o_inter matmul last
                        # so the cross-chunk state dependency lands as late
                        # as possible in the PE queue
                        p, hh = h // 2, h % 2
                        blk, pb = h // 2, (h % 2) * 64
                        o_slice = o_ps_pairs[p][:, hh, :]
                        vl = v_sb[:, vtb0, h * 128:(h + 1) * 128]
                        a_sb = a_sbs[h]
                        if ch > 0:
                            # inter first, jb1, then one merged full-width
                            # jb0 matmul carrying the single stop (last
                            # writer of both psum halves)
                            nc.tensor.matmul(
                                o_slice,
                                lhsT=s_sb[hh * 64:hh * 64 + 64, p,
                                          hh * 128:(hh + 1) * 128],
                                rhs=qs[pb:pb + 64, blk, :],
                                start=True, stop=False)
                            nc.tensor.matmul(o_slice[:, 128:256],
                                             lhsT=v_sb[:, vtb0 + 1,
                                                       h * 128:(h + 1) * 128],
                                             rhs=a_sb[:, 1, 128:256],
                                             start=False, stop=False)
                            nc.tensor.matmul(o_slice, lhsT=vl,
                                             rhs=a_sb[:, 0, :],
                                             start=False, stop=True)
                        else:
                            nc.tensor.matmul(o_slice[:, 0:128], lhsT=vl,
                                             rhs=a_sb[:, 0, 0:128],
                                             start=True, stop=True)
                            nc.tensor.matmul(o_slice[:, 128:256], lhsT=vl,
                                             rhs=a_sb[:, 0, 128:256],
                                             start=True, stop=False)
                            nc.tensor.matmul(o_slice[:, 128:256],
                                             lhsT=v_sb[:, vtb0 + 1,
                                                       h * 128:(h + 1) * 128],
                                             rhs=a_sb[:, 1, 128:256],
                                             start=False, stop=True)

                    def norm_front(p):
                        """scalar/vector front half of the norm chain."""
                        o_ps = o_ps_pairs[p]
                        o_flat = o_ps.rearrange('p a b -> p (a b)')
                        osc = nrm_pool.tile([128, 512], F32, tag="osc",
                                            name=f"osc{ch}_{p}")
                        nc.scalar.copy(osc, o_flat)
                        o2 = nrm_pool.tile([128, 512], BF16, tag="o2",
                                           name=f"o2{ch}_{p}")
                        nc.vector.tensor_mul(o2, osc, o_flat)
                        return osc, o2

                    def norm_mid(p, osc, o2):
                        """mean matmul + rsqrt + partition broadcast."""
                        mean_ps = ps_small.tile([1, 512], F32, tag="small",
                                                name=f"mean{ch}_{p}")
                        nc.tensor.matmul(mean_ps, lhsT=ones, rhs=o2,
                                         start=True, stop=True)
                        rsq1 = nrm_pool.tile([1, 512], F32, tag="rsq1",
                                             name=f"rsq{ch}_{p}")
                        nc.scalar.activation(rsq1, mean_ps,
                                             AF.Abs_reciprocal_sqrt,
                                             bias=epsb, scale=1.0 / DV)
                        rsqf = nrm_pool.tile([128, 512], F32, tag="rsqf",
                                             name=f"rsqf{ch}_{p}")
                        nc.gpsimd.partition_broadcast(rsqf, rsq1)
                        return rsqf

                    emit_at(0)
                    emit_at(1)

                    # state-update matmuls early: inputs (kdkv, v) are ready
                    # at chunk start, and the s_sb write then lands well
                    # before the next chunk's o_inter reads it
                    ds_ps = ps_small.tile([128, 2, C], F32, tag="small",
                                          name=f"ds{ch}")
                    for p in range(2):
                        for jb in range(2):
                            nc.tensor.matmul(
                                ds_ps[:, p, :],
                                lhsT=kdkv[:, jb, p * 128:(p + 1) * 128],
                                rhs=v_sb[:, vtb0 + jb, p * 256:(p + 1) * 256],
                                start=(jb == 0), stop=(jb == 1))

                    emit_o(0)
                    emit_at(2)
                    emit_o(1)            # pair 0 o_ps complete
                    nf0 = norm_front(0)
                    emit_at(3)
                    emit_o(2)
                    emit_o(HPC - 1)      # pair 1 o_ps complete

                    for p in range(2):
                        nc.vector.scalar_tensor_tensor(
                            out=s_sb[:, p, :],
                            in0=s_sb[:, p, :],
                            scalar=gcv[:, p:p + 1],
                            in1=ds_ps[:, p, :],
                            op0=mybir.AluOpType.mult,
                            op1=mybir.AluOpType.add)
                    nf1 = norm_front(1)
                    if ch == NCH - 1:
                        rsqf1 = norm_mid(1, *nf1)
                    rsqf0 = norm_mid(0, *nf0)

                    # deferred norm tail + output projection of older
                    # chunks (two-chunk deferral: og is never the limiter)
                    while pending_wo and not (ch == 1 and
                                               pending_wo[0][5] == 0):
                        pnf0, pnf1, prs0, prs1, pgsil, pch, pcc, pc0 = \
                            pending_wo.pop(0)
                        og_prev = [norm_tail_p(0, pnf0[0], prs0, pgsil,
                                               pch, pcc),
                                   norm_tail_p(1, pnf1[0], prs1, pgsil,
                                               pch, pcc)]
                        emit_wo(og_prev, pc0)
                    if ch != NCH - 1:
                        rsqf1 = norm_mid(1, *nf1)

                    if ch == NCH - 1:
                        og_pairs = [norm_tail_p(0, nf0[0], rsqf0, gsil,
                                                ch, cc),
                                    norm_tail_p(1, nf1[0], rsqf1, gsil,
                                                ch, cc)]
                        emit_wo(og_pairs, c0, last=True)
                    else:
                        pending_wo.append((nf0, nf1, rsqf0, rsqf1, gsil,
                                           ch, cc, c0))

    nc.finalize()
    return nc


def _host_tables(heads):
    """Per-core constant tables for a 4-head slice."""
    gam = (1.0 - 2.0 ** (-5.0 - np.arange(H, dtype=np.float64)))[heads]  # [4]

    # rope tables with retention decay folded in, feature-major [128, 4, T]
    # (cos m0, cos m1, sin m0, sin m1); rows permuted by P64 within each
    # 64-dim head block so rotate partners share a 32-partition quadrant
    inv = 10000.0 ** (-np.arange(0, DK, 2, dtype=np.float64) / DK)  # [32]
    t_idx = np.arange(T, dtype=np.float64)
    ang = np.outer(t_idx, inv)                      # [T, 32]
    cos_t, sin_t = np.cos(ang), np.sin(ang)         # [T, 32]
    i_in_chunk = (np.arange(T) % C).astype(np.float64)

    CSQ = np.empty((128, 4, T), np.float64)
    CSK = np.empty((128, 4, T), np.float64)
    for m in range(2):
        for p in range(128):
            f = m * 128 + p            # feature index within the 4-head slice
            hc = f // 64               # head-local index 0..3
            dd = P64[f % 64]           # original dim within the head
            idx = dd % 32
            sign = 1.0 if dd < 32 else -1.0
            dq = gam[hc] ** (i_in_chunk + 1.0)
            dk = gam[hc] ** (-i_in_chunk - 1.0)
            CSQ[p, m, :] = cos_t[:, idx] * dq
            CSQ[p, 2 + m, :] = sign * sin_t[:, idx] * dq
            CSK[p, m, :] = cos_t[:, idx] * dk
            CSK[p, 2 + m, :] = sign * sin_t[:, idx] * dk

    # full-tile A mask [128(j), 2(jb), 256(i)]: jb0 = [tri | ones]
    # (diagonal block then unmasked off-diagonal), jb1 = [zero | tri]
    # (the never-read strictly-upper quarter stays zeroed)
    j_idx = np.arange(128)
    tri = (j_idx[:, None] <= j_idx[None, :]).astype(np.float32)
    on = np.ones((128, 128), np.float32)
    JTt = np.stack([np.concatenate([tri, on], 1),
                    np.concatenate([0 * on, tri], 1)], axis=1)

    # gamma^C per k-token-major column (column c -> head c//64)
    GKCt = np.broadcast_to(
        np.repeat(gam ** C, 64)[None, :], (128, 256)).astype(np.float32)

    # gamma^C per state-pair row
    GCVt = np.empty((128, 2), np.float32)
    for p in range(2):
        GCVt[0:64, p] = gam[2 * p] ** C
        GCVt[64:128, p] = gam[2 * p + 1] ** C
    return CSQ, CSK, JTt, GKCt, GCVt


def _prepare_inputs(x, Wq, Wk, Wv, Wg, Wo, g_norm_w):
    x = np.asarray(x, np.float32)
    Wq = np.asarray(Wq, np.float32) * (DK ** -0.5)
    Wk = np.asarray(Wk, np.float32)
    Wv = np.asarray(Wv, np.float32)
    Wg = np.asarray(Wg, np.float32)
    Wo = np.asarray(Wo, np.float32)
    gw = np.asarray(g_norm_w, np.float32)

    in_maps = []
    for core in range(NCORES):
        b = core // 4
        hg = core % 4
        heads = np.arange(4 * hg, 4 * hg + 4)
        # q/k columns with the P64 row permutation applied per head
        qk_cols = np.concatenate(
            [h * DK + np.asarray(P64) for h in heads])
        vg_cols = np.concatenate([np.arange(h * DV, (h + 1) * DV) for h in heads])

        XTc = np.ascontiguousarray(x[b].T).astype(BF)
        WQc = np.ascontiguousarray(
            Wq[:, qk_cols].reshape(8, 128, 256).transpose(1, 0, 2)).astype(BF)
        WKc = np.ascontiguousarray(
            Wk[:, qk_cols].reshape(8, 128, 256).transpose(1, 0, 2)).astype(BF)
        WVc = np.ascontiguousarray(
            Wv[:, vg_cols].reshape(8, 128, 512).transpose(1, 0, 2)).astype(BF)
        WGc = np.ascontiguousarray(
            Wg[:, vg_cols].reshape(8, 128, 512).transpose(1, 0, 2)).astype(BF)
        WOc = np.ascontiguousarray(
            (Wo[vg_cols, :] * np.tile(gw, 4)[:, None])
            .reshape(4, 128, 1024).transpose(1, 0, 2)).astype(BF)

        CSQt, CSKt, JTt, GKCt, GCVt = _host_tables(heads)

        in_maps.append({
            "XT": XTc, "WQ": WQc, "WK": WKc, "WV": WVc, "WG": WGc, "WO": WOc,
            "CSQ": CSQt.astype(BF), "CSK": CSKt.astype(BF),
            "JT": JTt.astype(BF), "GKC": GKCt,
            "GCV": GCVt,
            "ONES": np.ones((128, 1), BF),
            "IDENT": np.eye(128, dtype=BF),
            "ZS": np.zeros((128, 2, C), BF),
        })
    return in_maps


def _run(in_maps, **kw):
    if "nc" not in _cache:
        _cache["nc"] = _build_program()
    return run_bass_kernel_spmd(_cache["nc"], in_maps,
                                core_ids=list(range(NCORES)), **kw)


def kernel(x, Wq, Wk, Wv, Wg, Wo, g_norm_w):
    in_maps = _prepare_inputs(x, Wq, Wk, Wv, Wg, Wo, g_norm_w)
    res = _run(in_maps)
    out = np.zeros((B, T, D), np.float32)
    for core in range(NCORES):
        out[core // 4] += res.results[core]["OUT"].astype(np.float32)
    return out
